# revision 1
# baseline (speedup 1.0000x reference)
"""Trainium2 Bass kernel for nn_ListenerModel (scatter_memory).

Strategy: pure data-parallel over batch (B=64 -> 8 rows/core), weights
replicated.  All matmuls are arranged so both operands load in natural
(row-major) layout; the big L=512-wide matmuls keep features on the
partition dim ([feat, L] outputs) so the chain
reps@W_emb -> @W_mm -> @W_a1 -> scores never needs an on-device
transpose of a large tensor.  Host pre-transposes reps / vc / sep once.
float32r operands get full PE rate at N=512 (plain fp32 is 4x slower).
DMAs are batched into multi-chunk 3D transfers to keep the Sync
sequencer's DIRECT2D descriptor generation off the critical path.
"""

import numpy as np
from contextlib import ExitStack

import concourse.bass as bass
import concourse.mybir as mybir
from concourse import bacc, tile
from concourse.bass_utils import run_bass_kernel_spmd

NCORES = 8
B, L, S, H = 64, 512, 6, 8
EMBED, HID, IMG, ATT = 1024, 512, 2048, 256
SIMG = S * IMG          # 12288
BC = B // NCORES        # 8 batch rows per core
BS = BC * S             # 48 (b,s) rows per core
BSH = BS * H            # 384
P = 128
FP = mybir.dt.float32
FPR = mybir.dt.float32r

KE = EMBED // P         # 8  k-chunks for EMBED contraction
KH = HID // P           # 4  k-chunks for HID contraction
KA = ATT // P           # 2  k-chunks for ATT contraction
KV = SIMG // P          # 96 k-chunks for the visual-context matmul
KI = IMG // P           # 16 k-chunks for separate-image projection
KBH = BSH // P          # 3  k-chunks for history averaging
NHT = HID // P          # 4  hid tiles
NAT = ATT // P          # 2  att tiles

WVB = 2                 # W_vis chunks per DMA
RPB = 4                 # reps chunks per DMA


def build_nc():
    nc = bacc.Bacc(None)

    # ---- DRAM I/O (per-core shapes); FPR = feeds a float32r matmul ----
    # 3D DRAM views are pre-chunked on the host: [n_chunks, 128, width]
    d_repsT = nc.dram_tensor("repsT", [BC, KE, P, L], FPR, kind="ExternalInput")
    d_vcT = nc.dram_tensor("vcT", [KV, P, BC], FPR, kind="ExternalInput")
    d_sepT = nc.dram_tensor("sepT", [KI, P, BS], FPR, kind="ExternalInput")
    d_hist = nc.dram_tensor("histf", [KBH, P, EMBED], FP, kind="ExternalInput")
    d_validW = nc.dram_tensor("validW", [KBH, P, BS], FP, kind="ExternalInput")
    d_Wvis = nc.dram_tensor("Wvis", [KV, P, HID], FPR, kind="ExternalInput")
    d_Wemb = nc.dram_tensor("Wemb", [KE, P, HID], FPR, kind="ExternalInput")
    d_Wmm = nc.dram_tensor("Wmm", [2 * KH, P, HID], FPR, kind="ExternalInput")
    d_Wsep = nc.dram_tensor("Wsep", [KI, P, HID], FPR, kind="ExternalInput")
    d_Wa1 = nc.dram_tensor("Wa1", [KH, P, ATT], FPR, kind="ExternalInput")
    d_Wa2 = nc.dram_tensor("Wa2", [KA, P, 1], FPR, kind="ExternalInput")
    d_bvis = nc.dram_tensor("bvis_row", [1, HID], FPR, kind="ExternalInput")
    d_bsep = nc.dram_tensor("bsep_row", [1, HID], FPR, kind="ExternalInput")
    d_bemb_row = nc.dram_tensor("bemb_row", [1, HID], FPR, kind="ExternalInput")
    d_ones = nc.dram_tensor("ones_row", [1, P], FPR, kind="ExternalInput")
    d_bemb_col = nc.dram_tensor("bemb_col", [NHT, P, 1], FP, kind="ExternalInput")
    d_bmm_col = nc.dram_tensor("bmm_col", [NHT, P, 1], FP, kind="ExternalInput")
    d_ba1_col = nc.dram_tensor("ba1_col", [NAT, P, 1], FP, kind="ExternalInput")
    d_mask = nc.dram_tensor("mask_row", [BC, L], FP, kind="ExternalInput")
    d_hh = nc.dram_tensor("hh_col", [BS, 1], FP, kind="ExternalInput")
    d_diagT = nc.dram_tensor("diagT", [BC, BS], FPR, kind="ExternalInput")
    d_ident = nc.dram_tensor("ident", [P, P], FP, kind="ExternalInput")
    d_out = nc.dram_tensor("out", [BS, 1], FP, kind="ExternalOutput")

    AFT = mybir.ActivationFunctionType
    AX = mybir.AxisListType

    with ExitStack() as ctx:
        tc = ctx.enter_context(tile.TileContext(nc))
        wres = ctx.enter_context(tc.tile_pool(name="wres", bufs=1))
        repsp = ctx.enter_context(tc.tile_pool(name="repsp", bufs=4))
        wvp = ctx.enter_context(tc.tile_pool(name="wvp", bufs=4))
        wsp = ctx.enter_context(tc.tile_pool(name="wsp", bufs=2))
        mm1p = ctx.enter_context(tc.tile_pool(name="mm1p", bufs=16))
        mm2p = ctx.enter_context(tc.tile_pool(name="mm2p", bufs=6))
        atthp = ctx.enter_context(tc.tile_pool(name="atthp", bufs=4))
        tmpp = ctx.enter_context(tc.tile_pool(name="tmpp", bufs=2))
        smp = ctx.enter_context(tc.tile_pool(name="smp", bufs=1))
        psA = ctx.enter_context(tc.tile_pool(name="psA", bufs=6, space="PSUM"))
        psB = ctx.enter_context(tc.tile_pool(name="psB", bufs=2, space="PSUM"))

        def wtile(shape, tag, dt=FP):
            return wres.tile(shape, dt, tag=tag, name=tag)

        def load(dst, src):
            nc.sync.dma_start(out=dst, in_=src)

        def body():
            # ---- streaming loads emitted first: W_vis + vcT get queue
            # priority so ctxmm unblocks as early as possible ----
            vct = wtile([P, KV, BC], "vct", FPR)          # all 96 chunks
            load(vct, d_vcT.rearrange("k p b -> p k b"))
            wv_tiles = []
            for i in range(KV // WVB):
                wv = wvp.tile([P, WVB, HID], FPR, tag="wv", name="wv")
                load(wv, d_Wvis[i * WVB:(i + 1) * WVB].rearrange(
                    "k p h -> p k h"))
                wv_tiles.append(wv)

            # ---- constants / small tensors ----
            ones = wtile([1, P], "ones", FPR)
            load(ones, d_ones[:, :])
            ident = wtile([P, P], "ident")
            load(ident, d_ident[:, :])
            hh_sb = wtile([BS, 1], "hh")
            load(hh_sb, d_hh[:, :])
            diagT_sb = wtile([BC, BS], "diagT", FPR)
            load(diagT_sb, d_diagT[:, :])
            bvis_sb = wtile([1, HID], "bvis", FPR)
            load(bvis_sb, d_bvis[:, :])
            bsep_sb = wtile([1, HID], "bsep", FPR)
            load(bsep_sb, d_bsep[:, :])
            bembr_sb = wtile([1, HID], "bembr", FPR)
            load(bembr_sb, d_bemb_row[:, :])
            bembc_sb = wtile([P, NHT], "bembc")
            load(bembc_sb, d_bemb_col.rearrange("h p one -> p (h one)"))
            ba1c_sb = wtile([P, NAT], "ba1c")
            load(ba1c_sb, d_ba1_col.rearrange("a p one -> p (a one)"))
            bmmc_sb = wtile([P, NHT], "bmmc")
            load(bmmc_sb, d_bmm_col.rearrange("h p one -> p (h one)"))
            wa2_sb = wtile([P, KA], "wa2", FPR)
            load(wa2_sb, d_Wa2.rearrange("k p one -> p (k one)"))
            validW_sb = wtile([P, KBH, BS], "validW")
            load(validW_sb, d_validW.rearrange("k p s -> p k s"))

            # ---- resident weights (single batched DMAs) ----
            wemb = wtile([P, KE, HID], "wemb", FPR)
            load(wemb, d_Wemb.rearrange("k p h -> p k h"))
            wmm = wtile([P, 2 * KH, HID], "wmm", FPR)
            load(wmm, d_Wmm.rearrange("k p h -> p k h"))
            wa1 = wtile([P, KH, ATT], "wa1", FPR)
            load(wa1, d_Wa1.rearrange("k p h -> p k h"))
            sepT_sb = wtile([P, KI, BS], "sepT", FPR)
            load(sepT_sb, d_sepT.rearrange("k p s -> p k s"))
            histf_sb = wtile([P, KBH, EMBED], "histf")
            load(histf_sb, d_hist.rearrange("k p e -> p k e"))

            # ---- visual context projection, interleaved with mm1 ----
            vc_psum = psB.tile([BC, HID], FP, tag="B", name="vc_psum")
            mm1_sb = {}

            def emit_vc_group(i):
                for j in range(WVB):
                    k = i * WVB + j
                    nc.tensor.matmul(vc_psum[:, :], vct[:, k, :],
                                     wv_tiles[i][:, j, :],
                                     start=(k == 0), stop=False)

            def emit_mm1_b(b):
                # mm1T[b]: [hid, L] = (W_emb.T @ reps[b].T), relu(+b_emb)
                rt = []
                for i in range(KE // RPB):
                    t = repsp.tile([P, RPB, L], FPR, tag="reps", name="rt")
                    load(t, d_repsT[b, i * RPB:(i + 1) * RPB].rearrange(
                        "k p l -> p k l"))
                    rt.append(t)
                for h in range(NHT):
                    ps = psA.tile([P, L], FP, tag="A", name="mm1ps")
                    for k in range(KE):
                        nc.tensor.matmul(
                            ps[:, :],
                            wemb[:, k, h * P:(h + 1) * P],
                            rt[k // RPB][:, k % RPB, :],
                            start=(k == 0), stop=(k == KE - 1))
                    t = mm1p.tile([P, L], FPR, tag="mm1", name=f"mm1_{b}_{h}")
                    nc.scalar.activation(t, ps[:, :], AFT.Relu,
                                         bias=bembc_sb[:, h:h + 1])
                    mm1_sb[(b, h)] = t

            # 48 vc chunk-groups interleaved with mm1 for b=0..3
            gpb = (KV // WVB) // 4  # 12 groups per b
            for b in range(4):
                for i in range(b * gpb, (b + 1) * gpb):
                    emit_vc_group(i)
                emit_mm1_b(b)

            # bias matmul: ones[1,8].T @ b_vis[1,512] adds b_vis to all rows
            nc.tensor.matmul(vc_psum[:, :], ones[:, :BC], bvis_sb[:, :],
                             start=False, stop=True)
            ctx_sb = wtile([BC, HID], "ctx_sb")
            nc.scalar.activation(ctx_sb, vc_psum[:, :], AFT.Relu)

            # transpose ctx [8, 512] -> ctxT [512, 8] via PE (4x [8,128])
            ctxT_sb = [wtile([P, BC], f"ctxT{h}", FPR) for h in range(NHT)]
            for h in range(NHT):
                tp = psB.tile([P, BC], FP, tag="B", name="ctxT_ps")
                nc.tensor.transpose(tp[:, :], ctx_sb[:, h * P:(h + 1) * P],
                                    ident[:BC, :BC])
                nc.scalar.activation(ctxT_sb[h], tp[:, :], AFT.Identity)

            # ctxmmb[h2] = W_mm_bot.T @ ctxT + b_mm   [128, 8] per hid2 tile
            ctxmmb_sb = [wtile([P, BC], f"ctxmmb{h}") for h in range(NHT)]
            for h2 in range(NHT):
                ps = psB.tile([P, BC], FP, tag="B", name="ctxmm_ps")
                for k in range(KH):
                    nc.tensor.matmul(ps[:, :],
                                     wmm[:, KH + k, h2 * P:(h2 + 1) * P],
                                     ctxT_sb[k][:, :],
                                     start=(k == 0), stop=(k == KH - 1))
                nc.scalar.activation(ctxmmb_sb[h2], ps[:, :], AFT.Identity,
                                     bias=bmmc_sb[:, h2:h2 + 1])

            # ---- separate images projection: sep[48, 512] ----
            sep_ps = psB.tile([BS, HID], FP, tag="B", name="sep_ps")
            for i in range(KI // 4):
                ws = wsp.tile([P, 4, HID], FPR, tag="ws", name="ws")
                load(ws, d_Wsep[i * 4:(i + 1) * 4].rearrange("k p h -> p k h"))
                for j in range(4):
                    k = i * 4 + j
                    nc.tensor.matmul(sep_ps[:, :], sepT_sb[:, k, :],
                                     ws[:, j, :],
                                     start=(k == 0), stop=False)
            nc.tensor.matmul(sep_ps[:, :], ones[:, :BS], bsep_sb[:, :],
                             start=False, stop=True)
            sep_sb = wtile([BS, HID], "sep_sb")
            nc.vector.tensor_copy(sep_sb, sep_ps[:, :])

            # ---- history: havgT[e,48] via block-diag valid-weight matmul ----
            havgT_sb = [wtile([P, BS], f"havgT{e}", FPR) for e in range(KE)]
            for e in range(KE):
                ps = psB.tile([P, BS], FP, tag="B", name="havg_ps")
                for k in range(KBH):
                    nc.tensor.matmul(ps[:, :],
                                     histf_sb[:, k, e * P:(e + 1) * P],
                                     validW_sb[:, k, :],
                                     start=(k == 0), stop=(k == KBH - 1))
                nc.scalar.activation(havgT_sb[e], ps[:, :], AFT.Identity)

            # hist_add[48, 512] = relu(havg @ W_emb + b_emb)
            ha_ps = psB.tile([BS, HID], FP, tag="B", name="ha_ps")
            for e in range(KE):
                nc.tensor.matmul(ha_ps[:, :], havgT_sb[e][:, :],
                                 wemb[:, e, :],
                                 start=(e == 0), stop=False)
            nc.tensor.matmul(ha_ps[:, :], ones[:, :BS], bembr_sb[:, :],
                             start=False, stop=True)
            hadd_sb = wtile([BS, HID], "hadd_sb")
            nc.scalar.activation(hadd_sb, ha_ps[:, :], AFT.Relu)

            # sep_final = sep + hh * hist_add
            sepfin_sb = wtile([BS, HID], "sepfin_sb")
            nc.vector.tensor_scalar_mul(sepfin_sb, hadd_sb, hh_sb)
            nc.vector.tensor_add(sepfin_sb, sepfin_sb, sep_sb)

            # ---- per-b: mm2 -> mm3 -> scores -> softmax -> attended ----
            attT_sb = [wtile([P, BC], f"attT{h}") for h in range(NHT)]
            for b in range(BC):
                if b < 4:
                    emit_mm1_b(b + 4)
                # mm2T[b]: [hid2, L] = relu(Wmm_top.T @ mm1T[b] + ctxmm[:,b])
                mm2t = []
                for h2 in range(NHT):
                    ps = psA.tile([P, L], FP, tag="A", name="mm2ps")
                    for k in range(KH):
                        nc.tensor.matmul(ps[:, :],
                                         wmm[:, k, h2 * P:(h2 + 1) * P],
                                         mm1_sb[(b, k)][:, :],
                                         start=(k == 0), stop=(k == KH - 1))
                    t = mm2p.tile([P, L], FPR, tag="mm2", name="mm2t")
                    nc.scalar.activation(t, ps[:, :], AFT.Relu,
                                         bias=ctxmmb_sb[h2][:, b:b + 1])
                    mm2t.append(t)
                # mm3: atthT [att, L] = tanh(W_a1.T @ mm2T + b_a1)
                atth = []
                for a in range(NAT):
                    ps = psA.tile([P, L], FP, tag="A", name="mm3ps")
                    for k in range(KH):
                        nc.tensor.matmul(ps[:, :],
                                         wa1[:, k, a * P:(a + 1) * P],
                                         mm2t[k][:, :],
                                         start=(k == 0), stop=(k == KH - 1))
                    t = atthp.tile([P, L], FPR, tag="atth", name="atht")
                    nc.scalar.activation(t, ps[:, :], AFT.Tanh,
                                         bias=ba1c_sb[:, a:a + 1])
                    atth.append(t)
                # scores row [1, L] = W_a2.T @ atthT (+mask incl. b_a2)
                sc_ps = psA.tile([1, L], FP, tag="A", name="scps")
                for k in range(KA):
                    nc.tensor.matmul(sc_ps[:, :], wa2_sb[:, k:k + 1],
                                     atth[k][:, :],
                                     start=(k == 0), stop=(k == KA - 1))
                mrow = smp.tile([1, L], FP, tag="mrow", name="mrow")
                load(mrow, d_mask[b:b + 1, :])
                att_row = smp.tile([1, L], FP, tag="attrow", name="att_row")
                nc.vector.tensor_add(att_row, sc_ps[:, :], mrow)
                # softmax over L (free axis), exp in place
                negmax = smp.tile([1, 1], FP, tag="negmax", name="negmax")
                nc.vector.reduce_max(negmax, att_row, axis=AX.X, negate=True)
                esum = smp.tile([1, 1], FP, tag="esum", name="esum")
                nc.scalar.activation(att_row, att_row, AFT.Exp, bias=negmax,
                                     accum_out=esum)
                rec = smp.tile([1, 1], FP, tag="rec", name="rec")
                nc.vector.reciprocal(rec, esum)
                # normalize + fp32r-round in one ACT copy
                wrow = smp.tile([1, L], FPR, tag="wrow", name="wrow")
                nc.scalar.activation(wrow, att_row, AFT.Copy, scale=rec)
                # broadcast w row to [128, L] via PE ones-product
                wb_ps = psA.tile([P, L], FP, tag="A", name="wbps")
                nc.tensor.matmul(wb_ps[:, :], ones[:, :], wrow[:, :],
                                 start=True, stop=True)
                # attended[:, b] = sum_l mm2T * w  (DVE mul + reduce)
                for h2 in range(NHT):
                    tmp = tmpp.tile([P, L], FP, tag="tmpa", name="tmpa")
                    nc.vector.tensor_mul(tmp, mm2t[h2].bitcast(FP)[:, :],
                                         wb_ps[:, :])
                    nc.vector.reduce_sum(attT_sb[h2][:, b:b + 1], tmp,
                                         axis=AX.X)

            # ---- attended rows [8, 512] via PE transpose of attT tiles ----
            attrows_sb = wtile([BC, HID], "attrows", FPR)
            for h in range(NHT):
                tp = psB.tile([BC, P], FP, tag="B", name="attrow_ps")
                nc.tensor.transpose(tp[:, :], attT_sb[h][:, :], ident[:, :])
                nc.scalar.activation(attrows_sb[:, h * P:(h + 1) * P],
                                     tp[:, :], AFT.Identity)

            # broadcast to [48, 512]: diagT.T @ attrows
            ab_ps = psB.tile([BS, HID], FP, tag="B", name="ab_ps")
            nc.tensor.matmul(ab_ps[:, :], diagT_sb[:, :], attrows_sb[:, :],
                             start=True, stop=True)
            # dot: out[48] = sum_hid sep_final * attended_bcast
            prod = tmpp.tile([BS, HID], FP, tag="tmpa", name="prod")
            nc.vector.tensor_mul(prod, sepfin_sb, ab_ps[:, :])
            out_sb = wtile([BS, 1], "out_sb")
            nc.vector.reduce_sum(out_sb, prod, axis=AX.X)
            nc.sync.dma_start(out=d_out[:, :], in_=out_sb)

        body()

    nc.compile()
    return nc


_NC_CACHE = None


def kernel(reps, separate_imgs, visual_context, masks, hist, hist_len,
           W_vis, b_vis, W_emb, b_emb, W_mm, b_mm, W_sep, b_sep,
           W_a1, b_a1, W_a2, b_a2):
    global _NC_CACHE
    f32 = np.float32

    def chunk(a):
        """[K, W] -> [K//128, 128, W] view."""
        a = np.ascontiguousarray(a, f32)
        return a.reshape(a.shape[0] // P, P, a.shape[1])

    reps = np.asarray(reps, f32)
    separate_imgs = np.asarray(separate_imgs, f32)
    visual_context = np.asarray(visual_context, f32)
    hist = np.asarray(hist, f32)
    hist_len = np.asarray(hist_len, np.int32)
    masks = np.asarray(masks)

    repsT = np.ascontiguousarray(reps.transpose(0, 2, 1))        # [B, EMBED, L]
    vcT = np.ascontiguousarray(visual_context.T)                 # [SIMG, B]
    mask_row = np.where(masks[:, :, 0], f32(-1e30), f32(0.0)) + f32(b_a2[0])
    ident = np.eye(P, dtype=f32)

    shared = {
        "Wvis": chunk(W_vis),
        "Wemb": chunk(W_emb),
        "Wmm": chunk(W_mm),
        "Wsep": chunk(W_sep),
        "Wa1": chunk(W_a1),
        "Wa2": chunk(np.ascontiguousarray(W_a2, f32).reshape(ATT, 1)),
        "bvis_row": np.ascontiguousarray(b_vis, f32).reshape(1, HID),
        "bsep_row": np.ascontiguousarray(b_sep, f32).reshape(1, HID),
        "bemb_row": np.ascontiguousarray(b_emb, f32).reshape(1, HID),
        "bemb_col": np.ascontiguousarray(b_emb, f32).reshape(NHT, P, 1),
        "bmm_col": np.ascontiguousarray(b_mm, f32).reshape(NHT, P, 1),
        "ba1_col": np.ascontiguousarray(b_a1, f32).reshape(NAT, P, 1),
        "ones_row": np.ones((1, P), f32),
        "ident": ident,
        "diagT": np.repeat(np.eye(BC, dtype=f32), S, axis=1).reshape(BC, BS),
    }

    in_maps = []
    for c in range(NCORES):
        sl = slice(c * BC, (c + 1) * BC)
        hl = hist_len[sl].reshape(BS)                            # [48]
        hvalid = (np.arange(H)[None, :] < hl[:, None]).astype(f32)
        hvalid /= np.maximum(hl, 1).astype(f32)[:, None]         # [48, H]
        validW = np.zeros((BSH, BS), f32)
        for bs in range(BS):
            validW[bs * H:(bs + 1) * H, bs] = hvalid[bs]
        m = {
            "repsT": np.ascontiguousarray(repsT[sl]).reshape(BC, KE, P, L),
            "vcT": chunk(np.ascontiguousarray(vcT[:, sl])),
            "sepT": chunk(np.ascontiguousarray(
                separate_imgs[sl].reshape(BS, IMG).T)),
            "histf": chunk(hist[sl].reshape(BSH, EMBED)),
            "validW": chunk(validW),
            "mask_row": np.ascontiguousarray(mask_row[sl]),
            "hh_col": (hl > 0).astype(f32).reshape(BS, 1),
        }
        m.update(shared)
        in_maps.append(m)

    if _NC_CACHE is None:
        _NC_CACHE = build_nc()
    res = run_bass_kernel_spmd(_NC_CACHE, in_maps, list(range(NCORES)))
    out = np.concatenate([r["out"].reshape(BC, S, 1) for r in res.results],
                         axis=0)
    return out.astype(f32)


if __name__ == "__main__":
    pass



# revision 7
# speedup vs baseline: 1.5506x; 1.5506x over previous
"""Trainium2 Bass kernel for nn_ListenerModel (scatter_memory).

Strategy: pure data-parallel over batch (B=64 -> 8 rows/core), weights
replicated.  v2 changes vs baseline:
  - all matmul operands bf16 (fp32r rhs streams at ~2 cycles/col on HW;
    bf16 streams 1/cycle) -> ~2x PE throughput, and DMA bytes halve.
  - host pre-lays every tensor out partition-major [128, F] so each DMA
    is one contiguous 2D transfer (cheap DIRECT2D descriptor gen).
  - DMA split across two queues: gpsimd (weights) + sync (activations).
  - visual-context matmuls (M=8) packed 4-wide into PE column groups
    via tile_position -> ~4x effective rate on that phase.
  - per-b softmax/attend chain software-pipelined one block behind the
    mm2/mm3/scores matmul stream so the PE queue never waits on it.
"""

import numpy as np
import ml_dtypes
from contextlib import ExitStack

import concourse.bass as bass
import concourse.mybir as mybir
from concourse import bacc, tile
from concourse.bass_utils import run_bass_kernel_spmd

NCORES = 8
B, L, S, H = 64, 512, 6, 8
EMBED, HID, IMG, ATT = 1024, 512, 2048, 256
SIMG = S * IMG          # 12288
BC = B // NCORES        # 8 batch rows per core
BS = BC * S             # 48 (b,s) rows per core
BSH = BS * H            # 384
P = 128
FP = mybir.dt.float32
BF = mybir.dt.bfloat16

KE = EMBED // P         # 8  k-chunks for EMBED contraction
KH = HID // P           # 4  k-chunks for HID contraction
KA = ATT // P           # 2  k-chunks for ATT contraction
KV = SIMG // P          # 96 k-chunks for the visual-context matmul
KI = IMG // P           # 16 k-chunks for separate-image projection
KBH = BSH // P          # 3  k-chunks for history averaging
NHT = HID // P          # 4  hid tiles
NAT = ATT // P          # 2  att tiles

WVB = 4                 # W_vis chunks per DMA / per packed vc group
NVG = KV // WVB         # 24 vc chunk groups


def build_nc():
    nc = bacc.Bacc(None)

    # ---- DRAM I/O; everything pre-laid-out partition-major on host ----
    d_reps = nc.dram_tensor("repsT", [BC, P, KE * L], BF, kind="ExternalInput")
    d_vcT = nc.dram_tensor("vcT", [P, KV * BC], BF, kind="ExternalInput")
    d_sepT = nc.dram_tensor("sepT", [P, KI * BS], BF, kind="ExternalInput")
    d_hist = nc.dram_tensor("histf", [P, KBH * EMBED], BF, kind="ExternalInput")
    d_validW = nc.dram_tensor("validW", [P, KBH * BS], BF, kind="ExternalInput")
    d_Wvis = nc.dram_tensor("Wvis", [NVG, P, WVB * HID], BF, kind="ExternalInput")
    d_Wemb = nc.dram_tensor("Wemb", [P, KE * HID], BF, kind="ExternalInput")
    d_Wmm = nc.dram_tensor("Wmm", [P, 2 * KH * HID], BF, kind="ExternalInput")
    d_Wsep = nc.dram_tensor("Wsep", [P, KI * HID], BF, kind="ExternalInput")
    d_Wa1 = nc.dram_tensor("Wa1", [P, KH * ATT], BF, kind="ExternalInput")
    d_Wa2 = nc.dram_tensor("Wa2", [P, KA], BF, kind="ExternalInput")
    d_bvis = nc.dram_tensor("bvis_row", [1, HID], BF, kind="ExternalInput")
    d_bsep = nc.dram_tensor("bsep_row", [1, HID], BF, kind="ExternalInput")
    d_bemb_row = nc.dram_tensor("bemb_row", [1, HID], BF, kind="ExternalInput")
    d_ones = nc.dram_tensor("ones_row", [1, P], BF, kind="ExternalInput")
    d_bemb_col = nc.dram_tensor("bemb_col", [P, NHT], FP, kind="ExternalInput")
    d_bmm_col = nc.dram_tensor("bmm_col", [P, NHT], FP, kind="ExternalInput")
    d_ba1_col = nc.dram_tensor("ba1_col", [P, NAT], FP, kind="ExternalInput")
    d_mask = nc.dram_tensor("mask_row", [BC, L], FP, kind="ExternalInput")
    d_hh = nc.dram_tensor("hh_col", [BS, 1], FP, kind="ExternalInput")
    d_diagT = nc.dram_tensor("diagT", [BC, BS], BF, kind="ExternalInput")
    d_ident = nc.dram_tensor("ident", [P, P], FP, kind="ExternalInput")
    d_gsel = nc.dram_tensor("gsel", [P, BC], BF, kind="ExternalInput")
    d_out = nc.dram_tensor("out", [BS, 1], FP, kind="ExternalOutput")

    AFT = mybir.ActivationFunctionType
    AX = mybir.AxisListType

    with ExitStack() as ctx:
        tc = ctx.enter_context(tile.TileContext(nc))
        wres = ctx.enter_context(tc.tile_pool(name="wres", bufs=1))
        repsp = ctx.enter_context(tc.tile_pool(name="repsp", bufs=4))
        wvp = ctx.enter_context(tc.tile_pool(name="wvp", bufs=4))
        mm1p = ctx.enter_context(tc.tile_pool(name="mm1p", bufs=32))
        mm2p = ctx.enter_context(tc.tile_pool(name="mm2p", bufs=12))
        atthp = ctx.enter_context(tc.tile_pool(name="atthp", bufs=4))
        tmpp = ctx.enter_context(tc.tile_pool(name="tmpp", bufs=3))
        wbtp = ctx.enter_context(tc.tile_pool(name="wbtp", bufs=2))
        smp = ctx.enter_context(tc.tile_pool(name="smp", bufs=2))
        # PSUM: 2 + 1 + 2 + 3 = 8 banks
        psM = ctx.enter_context(tc.tile_pool(name="psM", bufs=2, space="PSUM"))
        psV = ctx.enter_context(tc.tile_pool(name="psV", bufs=1, space="PSUM"))
        psE = ctx.enter_context(tc.tile_pool(name="psE", bufs=2, space="PSUM"))
        psB = ctx.enter_context(tc.tile_pool(name="psB", bufs=3, space="PSUM"))

        def wtile(shape, tag, dt=FP):
            return wres.tile(shape, dt, tag=tag, name=tag)

        def loadw(dst, src):
            nc.gpsimd.dma_start(out=dst, in_=src)

        def loads(dst, src):
            nc.sync.dma_start(out=dst, in_=src)

        def body():
            # =========== DMA queue W (gpsimd): weights ===========
            wemb = wtile([P, KE, HID], "wemb", BF)
            loadw(wemb, d_Wemb.rearrange("p (k h) -> p k h", k=KE))
            wv_tiles = []
            for g in range(NVG):
                wv = wvp.tile([P, WVB, HID], BF, tag="wv", name="wv")
                loadw(wv, d_Wvis[g].rearrange("p (k h) -> p k h", k=WVB))
                wv_tiles.append(wv)
            wmm = wtile([P, 2 * KH, HID], "wmm", BF)
            loadw(wmm, d_Wmm.rearrange("p (k h) -> p k h", k=2 * KH))
            wa1 = wtile([P, KH, ATT], "wa1", BF)
            loadw(wa1, d_Wa1.rearrange("p (k h) -> p k h", k=KH))
            wa2_sb = wtile([P, KA], "wa2", BF)
            loadw(wa2_sb, d_Wa2[:, :])
            ones = wtile([1, P], "ones", BF)
            loadw(ones, d_ones[:, :])
            ident = wtile([P, P], "ident")
            loadw(ident, d_ident[:, :])
            gsel = wtile([P, BC], "gsel", BF)
            loadw(gsel, d_gsel[:, :])
            bvis_sb = wtile([1, HID], "bvis", BF)
            loadw(bvis_sb, d_bvis[:, :])
            bsep_sb = wtile([1, HID], "bsep", BF)
            loadw(bsep_sb, d_bsep[:, :])
            bembr_sb = wtile([1, HID], "bembr", BF)
            loadw(bembr_sb, d_bemb_row[:, :])
            bembc_sb = wtile([P, NHT], "bembc")
            loadw(bembc_sb, d_bemb_col[:, :])
            ba1c_sb = wtile([P, NAT], "ba1c")
            loadw(ba1c_sb, d_ba1_col[:, :])
            bmmc_sb = wtile([P, NHT], "bmmc")
            loadw(bmmc_sb, d_bmm_col[:, :])

            # =========== DMA queue S (sync): activations ===========
            vct = wtile([P, KV, BC], "vct", BF)
            loads(vct, d_vcT.rearrange("p (k b) -> p k b", k=KV))
            reps_sb = []
            for b in range(BC):
                t = repsp.tile([P, KE, L], BF, tag="reps", name=f"reps{b}")
                loads(t, d_reps[b].rearrange("p (k l) -> p k l", k=KE))
                reps_sb.append(t)
            mrows = []
            for b in range(BC):
                m = wtile([1, L], f"mrow{b}")
                loads(m, d_mask[b:b + 1, :])
                mrows.append(m)
            hh_sb = wtile([BS, 1], "hh")
            loads(hh_sb, d_hh[:, :])
            diagT_sb = wtile([BC, BS], "diagT", BF)
            loads(diagT_sb, d_diagT[:, :])
            sepT_sb = wtile([P, KI, BS], "sepT", BF)
            loads(sepT_sb, d_sepT.rearrange("p (k s) -> p k s", k=KI))
            histf_sb = wtile([P, KBH, EMBED], "histf", BF)
            loads(histf_sb, d_hist.rearrange("p (k e) -> p k e", k=KBH))
            validW_sb = wtile([P, KBH, BS], "validW", BF)
            loads(validW_sb, d_validW.rearrange("p (k s) -> p k s", k=KBH))
            wsep_sb = wtile([P, KI, HID], "wsep", BF)
            loads(wsep_sb, d_Wsep.rearrange("p (k h) -> p k h", k=KI))

            # =========== phase 1: mm1 for all b, vc interleaved ===========
            # vc partials accumulate in 4 column groups of one PSUM bank:
            # chunk k -> col group k%4, psum rows [32*(k%4), +8)
            vc_ps = psV.tile([P, HID], FP, tag="V", name="vc_ps")
            # rows outside the 4 written slices are read (and zero-weighted)
            # by the gsel matmul later; clear so stale NaNs can't propagate
            nc.vector.memset(vc_ps[:, :], 0.0)

            def emit_vc_group(g):
                for j in range(WVB):
                    k = g * WVB + j
                    cg = 32 * j
                    nc.tensor.matmul(vc_ps[cg:cg + BC, :], vct[:, k, :],
                                     wv_tiles[g][:, j, :],
                                     start=(g == 0), stop=(g == NVG - 1),
                                     tile_position=(0, cg),
                                     skip_group_check=True)

            mm1_sb = {}
            vc_emitted = 0
            htile_no = 0

            def maybe_vc():
                # vc group g's weights arrive ~(2.9*g+9)us; PE h-tile t
                # finishes ~(1.7*t+6)us -> emit group g after h-tile
                # ceil(1.7*g+2)
                nonlocal vc_emitted
                while (vc_emitted < NVG
                       and htile_no >= int(1.7 * vc_emitted + 2)):
                    emit_vc_group(vc_emitted)
                    vc_emitted += 1

            for b in range(BC):
                for h in range(NHT):
                    ps = psM.tile([P, L], FP, tag="M", name="mm1ps")
                    for k in range(KE):
                        nc.tensor.matmul(
                            ps[:, :],
                            wemb[:, k, h * P:(h + 1) * P],
                            reps_sb[b][:, k, :],
                            start=(k == 0), stop=(k == KE - 1))
                    t = mm1p.tile([P, L], BF, tag="mm1", name=f"mm1_{b}_{h}")
                    nc.scalar.activation(t, ps[:, :], AFT.Relu,
                                         bias=bembc_sb[:, h:h + 1])
                    mm1_sb[(b, h)] = t
                    htile_no += 1
                    maybe_vc()
            while vc_emitted < NVG:
                emit_vc_group(vc_emitted)
                vc_emitted += 1

            # ---- vc finalize: cross-group sum via selector matmul ----
            vcp_sb = wtile([P, HID], "vcp_sb", BF)
            nc.scalar.activation(vcp_sb, vc_ps[:, :], AFT.Identity)
            ctx_ps = psE.tile([BC, HID], FP, tag="E", name="ctx_ps")
            nc.tensor.matmul(ctx_ps[:, :], gsel[:, :], vcp_sb[:, :],
                             start=True, stop=False)
            nc.tensor.matmul(ctx_ps[:, :], ones[:, :BC], bvis_sb[:, :],
                             start=False, stop=True)
            ctx_sb = wtile([BC, HID], "ctx_sb")
            nc.scalar.activation(ctx_sb, ctx_ps[:, :], AFT.Relu)

            # transpose ctx [8, 512] -> ctxT [512, 8] via PE (4x [8,128])
            ctxT_sb = [wtile([P, BC], f"ctxT{h}", BF) for h in range(NHT)]
            for h in range(NHT):
                tp = psE.tile([P, BC], FP, tag="E", name="ctxT_ps")
                nc.tensor.transpose(tp[:, :], ctx_sb[:, h * P:(h + 1) * P],
                                    ident[:BC, :BC])
                nc.scalar.activation(ctxT_sb[h], tp[:, :], AFT.Identity)

            # ctxmmb[h2] = W_mm_bot.T @ ctxT + b_mm   [128, 8] per h2
            ctxmmb_sb = [wtile([P, BC], f"ctxmmb{h}") for h in range(NHT)]
            for h2 in range(NHT):
                ps = psE.tile([P, BC], FP, tag="E", name="ctxmm_ps")
                for k in range(KH):
                    nc.tensor.matmul(ps[:, :],
                                     wmm[:, KH + k, h2 * P:(h2 + 1) * P],
                                     ctxT_sb[k][:, :],
                                     start=(k == 0), stop=(k == KH - 1))
                nc.scalar.activation(ctxmmb_sb[h2], ps[:, :], AFT.Identity,
                                     bias=bmmc_sb[:, h2:h2 + 1])

            # ---- separate images projection: sep[48, 512] ----
            sep_ps = psE.tile([BS, HID], FP, tag="E", name="sep_ps")
            for k in range(KI):
                nc.tensor.matmul(sep_ps[:, :], sepT_sb[:, k, :],
                                 wsep_sb[:, k, :],
                                 start=(k == 0), stop=False)
            nc.tensor.matmul(sep_ps[:, :], ones[:, :BS], bsep_sb[:, :],
                             start=False, stop=True)
            sep_sb = wtile([BS, HID], "sep_sb")
            nc.vector.tensor_copy(sep_sb, sep_ps[:, :])

            # ---- history: havgT[e][128, 48] then hist_add[48, 512] ----
            havgT_sb = [wtile([P, BS], f"havgT{e}", BF) for e in range(KE)]
            for e in range(KE):
                ps = psE.tile([P, BS], FP, tag="E", name="havg_ps")
                for k in range(KBH):
                    nc.tensor.matmul(ps[:, :],
                                     histf_sb[:, k, e * P:(e + 1) * P],
                                     validW_sb[:, k, :],
                                     start=(k == 0), stop=(k == KBH - 1))
                nc.scalar.activation(havgT_sb[e], ps[:, :], AFT.Identity)
            ha_ps = psE.tile([BS, HID], FP, tag="E", name="ha_ps")
            for e in range(KE):
                nc.tensor.matmul(ha_ps[:, :], havgT_sb[e][:, :],
                                 wemb[:, e, :],
                                 start=(e == 0), stop=False)
            nc.tensor.matmul(ha_ps[:, :], ones[:, :BS], bembr_sb[:, :],
                             start=False, stop=True)
            hadd_sb = wtile([BS, HID], "hadd_sb")
            nc.scalar.activation(hadd_sb, ha_ps[:, :], AFT.Relu)

            # sep_final = sep + hh * hist_add
            sepfin_sb = wtile([BS, HID], "sepfin_sb")
            nc.vector.tensor_scalar_mul(sepfin_sb, hadd_sb, hh_sb)
            nc.vector.tensor_add(sepfin_sb, sepfin_sb, sep_sb)

            # =========== phase 2: per-b pipeline ===========
            attT_sb = [wtile([P, BC], f"attT{h}") for h in range(NHT)]
            wrow_q = {}   # b -> wrow tile awaiting its wb matmul
            mm2_q = {}    # b -> mm2t list awaiting attended stage

            def emit_attend(b):
                # wb broadcast matmul + weighted-sum DVE for batch b
                wb_ps = psB.tile([P, L], FP, tag="B", name="wbps")
                nc.tensor.matmul(wb_ps[:, :], ones[:, :], wrow_q.pop(b)[:, :],
                                 start=True, stop=True)
                wbt = wbtp.tile([P, L], BF, tag="wbt", name="wbt")
                nc.vector.tensor_copy(wbt, wb_ps[:, :])
                for h2 in range(NHT):
                    tmp = tmpp.tile([P, L], BF, tag="tmpa", name="tmpa")
                    nc.vector.tensor_mul(tmp, mm2_q[b][h2][:, :], wbt)
                    nc.vector.reduce_sum(attT_sb[h2][:, b:b + 1], tmp,
                                         axis=AX.X)
                del mm2_q[b]

            for b in range(BC):
                # mm2T[b]: [hid2, L] = relu(Wmm_top.T @ mm1T[b] + ctxmm[:,b])
                mm2t = []
                for h2 in range(NHT):
                    ps = psB.tile([P, L], FP, tag="B", name="mm2ps")
                    for k in range(KH):
                        nc.tensor.matmul(ps[:, :],
                                         wmm[:, k, h2 * P:(h2 + 1) * P],
                                         mm1_sb[(b, k)][:, :],
                                         start=(k == 0), stop=(k == KH - 1))
                    t = mm2p.tile([P, L], BF, tag="mm2", name="mm2t")
                    nc.scalar.activation(t, ps[:, :], AFT.Relu,
                                         bias=ctxmmb_sb[h2][:, b:b + 1])
                    mm2t.append(t)
                mm2_q[b] = mm2t
                # mm3: atthT [att, L] = tanh(W_a1.T @ mm2T + b_a1)
                atth = []
                for a in range(NAT):
                    ps = psB.tile([P, L], FP, tag="B", name="mm3ps")
                    for k in range(KH):
                        nc.tensor.matmul(ps[:, :],
                                         wa1[:, k, a * P:(a + 1) * P],
                                         mm2t[k][:, :],
                                         start=(k == 0), stop=(k == KH - 1))
                    t = atthp.tile([P, L], BF, tag="atth", name="atht")
                    nc.scalar.activation(t, ps[:, :], AFT.Tanh,
                                         bias=ba1c_sb[:, a:a + 1])
                    atth.append(t)
                # scores row [1, L] = W_a2.T @ atthT
                sc_ps = psB.tile([1, L], FP, tag="B", name="scps")
                for k in range(KA):
                    nc.tensor.matmul(sc_ps[:, :], wa2_sb[:, k:k + 1],
                                     atth[k][:, :],
                                     start=(k == 0), stop=(k == KA - 1))
                att_row = smp.tile([1, L], FP, tag="attrow", name="att_row")
                nc.vector.tensor_add(att_row, sc_ps[:, :], mrows[b])
                # softmax over L (free axis), exp in place
                negmax = smp.tile([1, 1], FP, tag="negmax", name="negmax")
                nc.vector.reduce_max(negmax, att_row, axis=AX.X, negate=True)
                esum = smp.tile([1, 1], FP, tag="esum", name="esum")
                nc.scalar.activation(att_row, att_row, AFT.Exp, bias=negmax,
                                     accum_out=esum)
                rec = smp.tile([1, 1], FP, tag="rec", name="rec")
                nc.vector.reciprocal(rec, esum)
                wrow = smp.tile([1, L], BF, tag="wrow", name="wrow")
                nc.scalar.activation(wrow, att_row, AFT.Copy, scale=rec)
                wrow_q[b] = wrow
                # previous block's attended stage runs behind this block's
                # matmuls so the softmax latency never stalls the PE queue
                if b > 0:
                    emit_attend(b - 1)
            emit_attend(BC - 1)

            # ---- attended rows [8, 512] via PE transpose of attT tiles ----
            attrows_sb = wtile([BC, HID], "attrows", BF)
            for h in range(NHT):
                tp = psE.tile([BC, P], FP, tag="E", name="attrow_ps")
                nc.tensor.transpose(tp[:, :], attT_sb[h][:, :], ident[:, :])
                nc.scalar.activation(attrows_sb[:, h * P:(h + 1) * P],
                                     tp[:, :], AFT.Identity)

            # broadcast to [48, 512]: diagT.T @ attrows
            ab_ps = psB.tile([BS, HID], FP, tag="B", name="ab_ps")
            nc.tensor.matmul(ab_ps[:, :], diagT_sb[:, :], attrows_sb[:, :],
                             start=True, stop=True)
            # dot: out[48] = sum_hid sep_final * attended_bcast
            prod = wtile([BS, HID], "prod")
            nc.vector.tensor_mul(prod, sepfin_sb, ab_ps[:, :])
            out_sb = wtile([BS, 1], "out_sb")
            nc.vector.reduce_sum(out_sb, prod, axis=AX.X)
            nc.sync.dma_start(out=d_out[:, :], in_=out_sb)

        body()

    nc.compile()
    return nc


_NC_CACHE = None


def kernel(reps, separate_imgs, visual_context, masks, hist, hist_len,
           W_vis, b_vis, W_emb, b_emb, W_mm, b_mm, W_sep, b_sep,
           W_a1, b_a1, W_a2, b_a2):
    global _NC_CACHE
    f32 = np.float32
    bf16 = ml_dtypes.bfloat16

    def pm(a, kchunks):
        """[K, W] -> partition-major bf16 [128, kchunks*W]."""
        a = np.ascontiguousarray(a, f32)
        K, W = a.shape
        assert K == kchunks * P
        out = a.reshape(kchunks, P, W).transpose(1, 0, 2)
        return np.ascontiguousarray(out).astype(bf16).reshape(P, kchunks * W)

    reps = np.asarray(reps, f32)
    separate_imgs = np.asarray(separate_imgs, f32)
    visual_context = np.asarray(visual_context, f32)
    hist = np.asarray(hist, f32)
    hist_len = np.asarray(hist_len, np.int32)
    masks = np.asarray(masks)

    mask_row = np.where(masks[:, :, 0], f32(-1e30), f32(0.0)) + f32(b_a2[0])
    ident = np.eye(P, dtype=f32)
    gsel = np.zeros((P, BC), f32)
    for j in range(4):
        for i in range(BC):
            gsel[32 * j + i, i] = 1.0

    wvis_pm = np.ascontiguousarray(
        np.asarray(W_vis, f32).reshape(NVG, WVB, P, HID).transpose(0, 2, 1, 3)
    ).astype(bf16).reshape(NVG, P, WVB * HID)

    shared = {
        "Wvis": wvis_pm,
        "Wemb": pm(np.asarray(W_emb, f32), KE),
        "Wmm": pm(np.asarray(W_mm, f32), 2 * KH),
        "Wsep": pm(np.asarray(W_sep, f32), KI),
        "Wa1": pm(np.asarray(W_a1, f32), KH),
        "Wa2": pm(np.asarray(W_a2, f32).reshape(ATT, 1), KA).reshape(P, KA),
        "bvis_row": np.asarray(b_vis, f32).reshape(1, HID).astype(bf16),
        "bsep_row": np.asarray(b_sep, f32).reshape(1, HID).astype(bf16),
        "bemb_row": np.asarray(b_emb, f32).reshape(1, HID).astype(bf16),
        "bemb_col": np.ascontiguousarray(
            np.asarray(b_emb, f32).reshape(NHT, P).T),
        "bmm_col": np.ascontiguousarray(
            np.asarray(b_mm, f32).reshape(NHT, P).T),
        "ba1_col": np.ascontiguousarray(
            np.asarray(b_a1, f32).reshape(NAT, P).T),
        "ones_row": np.ones((1, P), bf16),
        "ident": ident,
        "gsel": gsel.astype(bf16),
        "diagT": np.repeat(np.eye(BC, dtype=f32), S, axis=1)
                   .reshape(BC, BS).astype(bf16),
    }

    # reps: [B, L, E] -> per-core [BC, 128, KE*L] partition-major
    repsT_pm = np.ascontiguousarray(
        reps.reshape(B, L, KE, P).transpose(0, 3, 2, 1)
    ).astype(bf16).reshape(B, P, KE * L)

    in_maps = []
    for c in range(NCORES):
        sl = slice(c * BC, (c + 1) * BC)
        hl = hist_len[sl].reshape(BS)                            # [48]
        hvalid = (np.arange(H)[None, :] < hl[:, None]).astype(f32)
        hvalid /= np.maximum(hl, 1).astype(f32)[:, None]         # [48, H]
        validW = np.zeros((BSH, BS), f32)
        for bs in range(BS):
            validW[bs * H:(bs + 1) * H, bs] = hvalid[bs]
        vcT = visual_context[sl].reshape(BC, KV, P).transpose(2, 1, 0)
        sepT = separate_imgs[sl].reshape(BS, KI, P).transpose(2, 1, 0)
        m = {
            "repsT": repsT_pm[sl],
            "vcT": np.ascontiguousarray(vcT).astype(bf16).reshape(P, KV * BC),
            "sepT": np.ascontiguousarray(sepT).astype(bf16)
                      .reshape(P, KI * BS),
            "histf": pm(hist[sl].reshape(BSH, EMBED), KBH),
            "validW": pm(validW, KBH),
            "mask_row": np.ascontiguousarray(mask_row[sl]),
            "hh_col": (hl > 0).astype(f32).reshape(BS, 1),
        }
        m.update(shared)
        in_maps.append(m)

    if _NC_CACHE is None:
        _NC_CACHE = build_nc()
    res = run_bass_kernel_spmd(_NC_CACHE, in_maps, list(range(NCORES)))
    out = np.concatenate([r["out"].reshape(BC, S, 1) for r in res.results],
                         axis=0)
    return out.astype(f32)


if __name__ == "__main__":
    pass


# revision 9
# speedup vs baseline: 1.9707x; 1.2709x over previous
"""Trainium2 Bass kernel for nn_ListenerModel (scatter_memory).

Strategy: pure data-parallel over batch (B=64 -> 8 rows/core), weights
replicated.  Key points:
  - ~50% of sequence positions are masked out (softmax weight exactly
    0), so the host compacts each batch's sequence to the unmasked
    positions (padded to LP; pad slots get -1e30 mask).  This halves
    the dominant mm1/mm2/mm3/scores matmul chain and the reps DMA.
    The math is exact: masked positions contribute nothing in the
    reference either.
  - all matmul operands bf16 (fp32r streams at ~2 cyc/col; bf16 1).
  - host pre-lays every tensor out partition-major [128, F] so DMAs
    are cheap contiguous transfers, split across two queues
    (sync + gpsimd) ordered by need-time.
  - visual-context matmuls (M=8) packed 4-wide into PE column groups.
  - per-b softmax/attend chain runs 1-2 blocks behind the matmul
    stream; outputs are produced per-b via attcol x sepfinT matmuls
    (no serial transpose tail).
"""

import numpy as np
import ml_dtypes
from contextlib import ExitStack

import concourse.bass as bass
import concourse.mybir as mybir
from concourse import bacc, tile
from concourse.bass_utils import run_bass_kernel_spmd

NCORES = 8
B, L, S, H = 64, 512, 6, 8
EMBED, HID, IMG, ATT = 1024, 512, 2048, 256
SIMG = S * IMG          # 12288
BC = B // NCORES        # 8 batch rows per core
BS = BC * S             # 48 (b,s) rows per core
BSH = BS * H            # 384
P = 128
FP = mybir.dt.float32
BF = mybir.dt.bfloat16

KE = EMBED // P         # 8  k-chunks for EMBED contraction
KH = HID // P           # 4  k-chunks for HID contraction
KA = ATT // P           # 2  k-chunks for ATT contraction
KV = SIMG // P          # 96 k-chunks for the visual-context matmul
KI = IMG // P           # 16 k-chunks for separate-image projection
KBH = BSH // P          # 3  k-chunks for history averaging
NHT = HID // P          # 4  hid tiles
NAT = ATT // P          # 2  att tiles

WVB = 4                 # W_vis chunks per DMA / per packed vc group
NVG = KV // WVB         # 24 vc chunk groups
LP = 320                # compacted sequence capacity (max kept ~284)


def build_nc(lp):
    nc = bacc.Bacc(None)

    # ---- DRAM I/O; everything pre-laid-out partition-major on host ----
    d_reps = nc.dram_tensor("repsT", [BC, P, KE * lp], BF, kind="ExternalInput")
    d_vcT = nc.dram_tensor("vcT", [P, KV * BC], BF, kind="ExternalInput")
    d_sepT = nc.dram_tensor("sepT", [P, KI * BS], BF, kind="ExternalInput")
    d_hist = nc.dram_tensor("histf", [P, KBH * EMBED], BF, kind="ExternalInput")
    d_validW = nc.dram_tensor("validW", [P, KBH * BS], BF, kind="ExternalInput")
    d_Wvis = nc.dram_tensor("Wvis", [NVG, P, WVB * HID], BF, kind="ExternalInput")
    d_Wemb = nc.dram_tensor("Wemb", [P, KE * HID], BF, kind="ExternalInput")
    d_Wmm = nc.dram_tensor("Wmm", [P, 2 * KH * HID], BF, kind="ExternalInput")
    d_Wsep = nc.dram_tensor("Wsep", [P, KI * HID], BF, kind="ExternalInput")
    d_Wa1 = nc.dram_tensor("Wa1", [P, KH * ATT], BF, kind="ExternalInput")
    d_Wa2 = nc.dram_tensor("Wa2", [P, KA], BF, kind="ExternalInput")
    d_bvis = nc.dram_tensor("bvis_row", [1, HID], BF, kind="ExternalInput")
    d_bsep = nc.dram_tensor("bsep_row", [1, HID], BF, kind="ExternalInput")
    d_bemb_row = nc.dram_tensor("bemb_row", [1, HID], BF, kind="ExternalInput")
    d_ones = nc.dram_tensor("ones_row", [1, P], BF, kind="ExternalInput")
    d_bemb_col = nc.dram_tensor("bemb_col", [P, NHT], FP, kind="ExternalInput")
    d_bmm_col = nc.dram_tensor("bmm_col", [P, NHT], FP, kind="ExternalInput")
    d_ba1_col = nc.dram_tensor("ba1_col", [P, NAT], FP, kind="ExternalInput")
    d_mask = nc.dram_tensor("mask_row", [BC, lp], FP, kind="ExternalInput")
    d_hh = nc.dram_tensor("hh_col", [BS, 1], FP, kind="ExternalInput")
    d_ident = nc.dram_tensor("ident", [P, P], FP, kind="ExternalInput")
    d_gsel = nc.dram_tensor("gsel", [P, BC], BF, kind="ExternalInput")
    d_out = nc.dram_tensor("out", [1, BS], FP, kind="ExternalOutput")

    AFT = mybir.ActivationFunctionType
    AX = mybir.AxisListType

    with ExitStack() as ctx:
        tc = ctx.enter_context(tile.TileContext(nc))
        wres = ctx.enter_context(tc.tile_pool(name="wres", bufs=1))
        repsp = ctx.enter_context(tc.tile_pool(name="repsp", bufs=3))
        wvp = ctx.enter_context(tc.tile_pool(name="wvp", bufs=4))
        mm1p = ctx.enter_context(tc.tile_pool(name="mm1p", bufs=32))
        mm2p = ctx.enter_context(tc.tile_pool(name="mm2p", bufs=12))
        atthp = ctx.enter_context(tc.tile_pool(name="atthp", bufs=4))
        tmpp = ctx.enter_context(tc.tile_pool(name="tmpp", bufs=3))
        wbtp = ctx.enter_context(tc.tile_pool(name="wbtp", bufs=2))
        attcp = ctx.enter_context(tc.tile_pool(name="attcp", bufs=8))
        smp = ctx.enter_context(tc.tile_pool(name="smp", bufs=3))
        # PSUM: 2 + 1 + 2 + 3 = 8 banks
        psM = ctx.enter_context(tc.tile_pool(name="psM", bufs=2, space="PSUM"))
        psV = ctx.enter_context(tc.tile_pool(name="psV", bufs=1, space="PSUM"))
        psE = ctx.enter_context(tc.tile_pool(name="psE", bufs=2, space="PSUM"))
        psB = ctx.enter_context(tc.tile_pool(name="psB", bufs=3, space="PSUM"))

        def wtile(shape, tag, dt=FP):
            return wres.tile(shape, dt, tag=tag, name=tag)

        def loadw(dst, src):
            nc.gpsimd.dma_start(out=dst, in_=src)

        def loads(dst, src):
            nc.sync.dma_start(out=dst, in_=src)

        def body():
            # ===== DMA queue S (sync): start-critical + activations =====
            wemb = wtile([P, KE, HID], "wemb", BF)
            loads(wemb, d_Wemb.rearrange("p (k h) -> p k h", k=KE))
            vct = wtile([P, KV, BC], "vct", BF)
            loads(vct, d_vcT.rearrange("p (k b) -> p k b", k=KV))
            reps_sb = []
            for b in range(BC):
                t = repsp.tile([P, KE, lp], BF, tag="reps", name=f"reps{b}")
                loads(t, d_reps[b].rearrange("p (k l) -> p k l", k=KE))
                reps_sb.append(t)
            mrows = []
            for b in range(BC):
                m = wtile([1, lp], f"mrow{b}")
                loads(m, d_mask[b:b + 1, :])
                mrows.append(m)
            sepT_sb = wtile([P, KI, BS], "sepT", BF)
            loads(sepT_sb, d_sepT.rearrange("p (k s) -> p k s", k=KI))
            histf_sb = wtile([P, KBH, EMBED], "histf", BF)
            loads(histf_sb, d_hist.rearrange("p (k e) -> p k e", k=KBH))
            validW_sb = wtile([P, KBH, BS], "validW", BF)
            loads(validW_sb, d_validW.rearrange("p (k s) -> p k s", k=KBH))
            wsep_sb = wtile([P, KI, HID], "wsep", BF)
            loads(wsep_sb, d_Wsep.rearrange("p (k h) -> p k h", k=KI))
            hh_sb = wtile([BS, 1], "hh")
            loads(hh_sb, d_hh[:, :])

            # ===== DMA queue W (gpsimd): W_vis stream + later weights =====
            wv_tiles = []
            for g in range(NVG):
                wv = wvp.tile([P, WVB, HID], BF, tag="wv", name="wv")
                loadw(wv, d_Wvis[g].rearrange("p (k h) -> p k h", k=WVB))
                wv_tiles.append(wv)
            wmm = wtile([P, 2 * KH, HID], "wmm", BF)
            loadw(wmm, d_Wmm.rearrange("p (k h) -> p k h", k=2 * KH))
            wa1 = wtile([P, KH, ATT], "wa1", BF)
            loadw(wa1, d_Wa1.rearrange("p (k h) -> p k h", k=KH))
            wa2_sb = wtile([P, KA], "wa2", BF)
            loadw(wa2_sb, d_Wa2[:, :])
            ones = wtile([1, P], "ones", BF)
            loadw(ones, d_ones[:, :])
            ident = wtile([P, P], "ident")
            loadw(ident, d_ident[:, :])
            gsel = wtile([P, BC], "gsel", BF)
            loadw(gsel, d_gsel[:, :])
            bvis_sb = wtile([1, HID], "bvis", BF)
            loadw(bvis_sb, d_bvis[:, :])
            bsep_sb = wtile([1, HID], "bsep", BF)
            loadw(bsep_sb, d_bsep[:, :])
            bembr_sb = wtile([1, HID], "bembr", BF)
            loadw(bembr_sb, d_bemb_row[:, :])
            bembc_sb = wtile([P, NHT], "bembc")
            loadw(bembc_sb, d_bemb_col[:, :])
            ba1c_sb = wtile([P, NAT], "ba1c")
            loadw(ba1c_sb, d_ba1_col[:, :])
            bmmc_sb = wtile([P, NHT], "bmmc")
            loadw(bmmc_sb, d_bmm_col[:, :])

            # =========== phase 1: mm1 for all b, vc interleaved ===========
            vc_ps = psV.tile([P, HID], FP, tag="V", name="vc_ps")
            # rows outside the 4 written col-group slices are read (and
            # zero-weighted) by the gsel matmul; clear stale data once
            nc.vector.memset(vc_ps[:, :], 0.0)

            def emit_vc_group(g):
                for j in range(WVB):
                    k = g * WVB + j
                    cg = 32 * j
                    nc.tensor.matmul(vc_ps[cg:cg + BC, :], vct[:, k, :],
                                     wv_tiles[g][:, j, :],
                                     start=(g == 0), stop=(g == NVG - 1),
                                     tile_position=(0, cg),
                                     skip_group_check=True)

            mm1_sb = {}
            vc_emitted = 0
            htile_no = 0

            def maybe_vc():
                # wvis group g lands ~(2.6*g+4)us; mm1 h-tile t ends
                # ~(1.2*t+6)us -> emit group g after h-tile ~2.2g
                nonlocal vc_emitted
                while (vc_emitted < NVG
                       and htile_no >= int(2.2 * vc_emitted + 1)):
                    emit_vc_group(vc_emitted)
                    vc_emitted += 1

            for b in range(BC):
                for h in range(NHT):
                    ps = psM.tile([P, lp], FP, tag="M", name="mm1ps")
                    for k in range(KE):
                        nc.tensor.matmul(
                            ps[:, :],
                            wemb[:, k, h * P:(h + 1) * P],
                            reps_sb[b][:, k, :],
                            start=(k == 0), stop=(k == KE - 1))
                    t = mm1p.tile([P, lp], BF, tag="mm1", name=f"mm1_{b}_{h}")
                    nc.scalar.activation(t, ps[:, :], AFT.Relu,
                                         bias=bembc_sb[:, h:h + 1])
                    mm1_sb[(b, h)] = t
                    htile_no += 1
                    maybe_vc()

            # ---- fillers while the W_vis tail streams in ----
            # separate images projection: sep[48, 512]
            sep_ps = psE.tile([BS, HID], FP, tag="E", name="sep_ps")
            for k in range(KI):
                nc.tensor.matmul(sep_ps[:, :], sepT_sb[:, k, :],
                                 wsep_sb[:, k, :],
                                 start=(k == 0), stop=False)
            nc.tensor.matmul(sep_ps[:, :], ones[:, :BS], bsep_sb[:, :],
                             start=False, stop=True)
            sep_sb = wtile([BS, HID], "sep_sb")
            nc.vector.tensor_copy(sep_sb, sep_ps[:, :])

            # history: havgT[e][128, 48] then hist_add[48, 512]
            havgT_sb = [wtile([P, BS], f"havgT{e}", BF) for e in range(KE)]
            for e in range(KE):
                ps = psE.tile([P, BS], FP, tag="E", name="havg_ps")
                for k in range(KBH):
                    nc.tensor.matmul(ps[:, :],
                                     histf_sb[:, k, e * P:(e + 1) * P],
                                     validW_sb[:, k, :],
                                     start=(k == 0), stop=(k == KBH - 1))
                nc.scalar.activation(havgT_sb[e], ps[:, :], AFT.Identity)
            ha_ps = psE.tile([BS, HID], FP, tag="E", name="ha_ps")
            for e in range(KE):
                nc.tensor.matmul(ha_ps[:, :], havgT_sb[e][:, :],
                                 wemb[:, e, :],
                                 start=(e == 0), stop=False)
            nc.tensor.matmul(ha_ps[:, :], ones[:, :BS], bembr_sb[:, :],
                             start=False, stop=True)
            hadd_sb = wtile([BS, HID], "hadd_sb")
            nc.scalar.activation(hadd_sb, ha_ps[:, :], AFT.Relu)

            # sep_final = sep + hh * hist_add, then transpose to [hid, 48]
            sepfin_sb = wtile([BS, HID], "sepfin_sb")
            nc.vector.tensor_scalar_mul(sepfin_sb, hadd_sb, hh_sb)
            nc.vector.tensor_add(sepfin_sb, sepfin_sb, sep_sb)
            sepfinT = [wtile([P, BS], f"sepfinT{h}", BF) for h in range(NHT)]
            for h in range(NHT):
                tp = psE.tile([P, BS], FP, tag="E", name="sfT_ps")
                nc.tensor.transpose(tp[:, :],
                                    sepfin_sb[:, h * P:(h + 1) * P],
                                    ident[:BS, :BS])
                nc.scalar.activation(sepfinT[h], tp[:, :], AFT.Identity)

            # ---- vc leftovers, then ctx chain ----
            while vc_emitted < NVG:
                emit_vc_group(vc_emitted)
                vc_emitted += 1
            vcp_sb = wtile([P, HID], "vcp_sb", BF)
            nc.scalar.activation(vcp_sb, vc_ps[:, :], AFT.Identity)
            ctx_ps = psE.tile([BC, HID], FP, tag="E", name="ctx_ps")
            nc.tensor.matmul(ctx_ps[:, :], gsel[:, :], vcp_sb[:, :],
                             start=True, stop=False)
            nc.tensor.matmul(ctx_ps[:, :], ones[:, :BC], bvis_sb[:, :],
                             start=False, stop=True)
            ctx_sb = wtile([BC, HID], "ctx_sb")
            nc.scalar.activation(ctx_sb, ctx_ps[:, :], AFT.Relu)

            ctxT_sb = [wtile([P, BC], f"ctxT{h}", BF) for h in range(NHT)]
            for h in range(NHT):
                tp = psE.tile([P, BC], FP, tag="E", name="ctxT_ps")
                nc.tensor.transpose(tp[:, :], ctx_sb[:, h * P:(h + 1) * P],
                                    ident[:BC, :BC])
                nc.scalar.activation(ctxT_sb[h], tp[:, :], AFT.Identity)

            ctxmmb_sb = [wtile([P, BC], f"ctxmmb{h}") for h in range(NHT)]
            for h2 in range(NHT):
                ps = psE.tile([P, BC], FP, tag="E", name="ctxmm_ps")
                for k in range(KH):
                    nc.tensor.matmul(ps[:, :],
                                     wmm[:, KH + k, h2 * P:(h2 + 1) * P],
                                     ctxT_sb[k][:, :],
                                     start=(k == 0), stop=(k == KH - 1))
                nc.scalar.activation(ctxmmb_sb[h2], ps[:, :], AFT.Identity,
                                     bias=bmmc_sb[:, h2:h2 + 1])

            # =========== phase 2: per-b pipeline ===========
            outrow = wtile([1, BS], "outrow")
            wrow_q = {}
            mm2_q = {}

            def emit_attend(b):
                # broadcast softmax row, weight mm2, reduce to attcol[h2],
                # then out[6b:6b+6] = sum_h2 attcol.T @ sepfinT
                wb_ps = psB.tile([P, lp], FP, tag="B", name="wbps")
                nc.tensor.matmul(wb_ps[:, :], ones[:, :], wrow_q.pop(b)[:, :],
                                 start=True, stop=True)
                wbt = wbtp.tile([P, lp], BF, tag="wbt", name="wbt")
                nc.vector.tensor_copy(wbt, wb_ps[:, :])
                attc = []
                for h2 in range(NHT):
                    tmp = tmpp.tile([P, lp], BF, tag="tmpa", name="tmpa")
                    nc.vector.tensor_mul(tmp, mm2_q[b][h2][:, :], wbt)
                    ac = attcp.tile([P, 1], BF, tag="attc", name="attc")
                    with nc.allow_low_precision(
                            reason="attended col consumed by bf16 matmul"):
                        nc.vector.reduce_sum(ac, tmp, axis=AX.X)
                    attc.append(ac)
                del mm2_q[b]
                o_ps = psE.tile([1, BS], FP, tag="E", name="o_ps")
                for h2 in range(NHT):
                    nc.tensor.matmul(o_ps[:, :], attc[h2][:, :],
                                     sepfinT[h2][:, :],
                                     start=(h2 == 0), stop=(h2 == NHT - 1))
                nc.vector.tensor_copy(outrow[0:1, S * b:S * (b + 1)],
                                      o_ps[0:1, S * b:S * (b + 1)])

            for b in range(BC):
                # mm2T[b]: [hid2, LP] = relu(Wmm_top.T @ mm1T[b] + ctxmm[:,b])
                mm2t = []
                for h2 in range(NHT):
                    ps = psB.tile([P, lp], FP, tag="B", name="mm2ps")
                    for k in range(KH):
                        nc.tensor.matmul(ps[:, :],
                                         wmm[:, k, h2 * P:(h2 + 1) * P],
                                         mm1_sb[(b, k)][:, :],
                                         start=(k == 0), stop=(k == KH - 1))
                    t = mm2p.tile([P, lp], BF, tag="mm2", name="mm2t")
                    nc.scalar.activation(t, ps[:, :], AFT.Relu,
                                         bias=ctxmmb_sb[h2][:, b:b + 1])
                    mm2t.append(t)
                mm2_q[b] = mm2t
                # mm3: atthT [att, LP] = tanh(W_a1.T @ mm2T + b_a1)
                atth = []
                for a in range(NAT):
                    ps = psB.tile([P, lp], FP, tag="B", name="mm3ps")
                    for k in range(KH):
                        nc.tensor.matmul(ps[:, :],
                                         wa1[:, k, a * P:(a + 1) * P],
                                         mm2t[k][:, :],
                                         start=(k == 0), stop=(k == KH - 1))
                    t = atthp.tile([P, lp], BF, tag="atth", name="atht")
                    nc.scalar.activation(t, ps[:, :], AFT.Tanh,
                                         bias=ba1c_sb[:, a:a + 1])
                    atth.append(t)
                # scores row [1, LP] = W_a2.T @ atthT
                sc_ps = psB.tile([1, lp], FP, tag="B", name="scps")
                for k in range(KA):
                    nc.tensor.matmul(sc_ps[:, :], wa2_sb[:, k:k + 1],
                                     atth[k][:, :],
                                     start=(k == 0), stop=(k == KA - 1))
                att_row = smp.tile([1, lp], FP, tag="attrow", name="att_row")
                nc.vector.tensor_add(att_row, sc_ps[:, :], mrows[b])
                negmax = smp.tile([1, 1], FP, tag="negmax", name="negmax")
                nc.vector.reduce_max(negmax, att_row, axis=AX.X, negate=True)
                esum = smp.tile([1, 1], FP, tag="esum", name="esum")
                nc.scalar.activation(att_row, att_row, AFT.Exp, bias=negmax,
                                     accum_out=esum)
                rec = smp.tile([1, 1], FP, tag="rec", name="rec")
                nc.vector.reciprocal(rec, esum)
                wrow = smp.tile([1, lp], BF, tag="wrow", name="wrow")
                nc.scalar.activation(wrow, att_row, AFT.Copy, scale=rec)
                wrow_q[b] = wrow
                if b > 0:
                    emit_attend(b - 1)
            emit_attend(BC - 1)

            nc.sync.dma_start(out=d_out[:, :], in_=outrow)

        body()

    nc.compile()
    return nc


_NC_CACHE = {}


def kernel(reps, separate_imgs, visual_context, masks, hist, hist_len,
           W_vis, b_vis, W_emb, b_emb, W_mm, b_mm, W_sep, b_sep,
           W_a1, b_a1, W_a2, b_a2):
    f32 = np.float32
    bf16 = ml_dtypes.bfloat16

    def pm(a, kchunks):
        """[K, W] -> partition-major bf16 [128, kchunks*W]."""
        a = np.ascontiguousarray(a, f32)
        K, W = a.shape
        assert K == kchunks * P
        out = a.reshape(kchunks, P, W).transpose(1, 0, 2)
        return np.ascontiguousarray(out).astype(bf16).reshape(P, kchunks * W)

    reps = np.asarray(reps, f32)
    separate_imgs = np.asarray(separate_imgs, f32)
    visual_context = np.asarray(visual_context, f32)
    hist = np.asarray(hist, f32)
    hist_len = np.asarray(hist_len, np.int32)
    masks = np.asarray(masks)[:, :, 0]          # True -> masked out

    # ---- compact each batch's sequence to its unmasked positions ----
    keep_counts = (~masks).sum(axis=1)
    lp = LP if keep_counts.max() <= LP else L
    reps_c = np.zeros((B, lp, EMBED), f32)
    mask_c = np.full((B, lp), f32(-1e30)) + f32(b_a2[0])
    for b in range(B):
        idx = np.nonzero(~masks[b])[0]
        reps_c[b, :len(idx)] = reps[b, idx]
        mask_c[b, :len(idx)] = f32(b_a2[0])

    ident = np.eye(P, dtype=f32)
    gsel = np.zeros((P, BC), f32)
    for j in range(4):
        for i in range(BC):
            gsel[32 * j + i, i] = 1.0

    wvis_pm = np.ascontiguousarray(
        np.asarray(W_vis, f32).reshape(NVG, WVB, P, HID).transpose(0, 2, 1, 3)
    ).astype(bf16).reshape(NVG, P, WVB * HID)

    shared = {
        "Wvis": wvis_pm,
        "Wemb": pm(np.asarray(W_emb, f32), KE),
        "Wmm": pm(np.asarray(W_mm, f32), 2 * KH),
        "Wsep": pm(np.asarray(W_sep, f32), KI),
        "Wa1": pm(np.asarray(W_a1, f32), KH),
        "Wa2": pm(np.asarray(W_a2, f32).reshape(ATT, 1), KA).reshape(P, KA),
        "bvis_row": np.asarray(b_vis, f32).reshape(1, HID).astype(bf16),
        "bsep_row": np.asarray(b_sep, f32).reshape(1, HID).astype(bf16),
        "bemb_row": np.asarray(b_emb, f32).reshape(1, HID).astype(bf16),
        "bemb_col": np.ascontiguousarray(
            np.asarray(b_emb, f32).reshape(NHT, P).T),
        "bmm_col": np.ascontiguousarray(
            np.asarray(b_mm, f32).reshape(NHT, P).T),
        "ba1_col": np.ascontiguousarray(
            np.asarray(b_a1, f32).reshape(NAT, P).T),
        "ones_row": np.ones((1, P), bf16),
        "ident": ident,
        "gsel": gsel.astype(bf16),
    }

    # reps: [B, lp, E] -> per-b [128, KE*lp] partition-major
    repsT_pm = np.ascontiguousarray(
        reps_c.reshape(B, lp, KE, P).transpose(0, 3, 2, 1)
    ).astype(bf16).reshape(B, P, KE * lp)

    in_maps = []
    for c in range(NCORES):
        sl = slice(c * BC, (c + 1) * BC)
        hl = hist_len[sl].reshape(BS)                            # [48]
        hvalid = (np.arange(H)[None, :] < hl[:, None]).astype(f32)
        hvalid /= np.maximum(hl, 1).astype(f32)[:, None]         # [48, H]
        validW = np.zeros((BSH, BS), f32)
        for bs in range(BS):
            validW[bs * H:(bs + 1) * H, bs] = hvalid[bs]
        vcT = visual_context[sl].reshape(BC, KV, P).transpose(2, 1, 0)
        sepT = separate_imgs[sl].reshape(BS, KI, P).transpose(2, 1, 0)
        m = {
            "repsT": repsT_pm[sl],
            "vcT": np.ascontiguousarray(vcT).astype(bf16).reshape(P, KV * BC),
            "sepT": np.ascontiguousarray(sepT).astype(bf16)
                      .reshape(P, KI * BS),
            "histf": pm(hist[sl].reshape(BSH, EMBED), KBH),
            "validW": pm(validW, KBH),
            "mask_row": np.ascontiguousarray(mask_c[sl]),
            "hh_col": (hl > 0).astype(f32).reshape(BS, 1),
        }
        m.update(shared)
        in_maps.append(m)

    if lp not in _NC_CACHE:
        _NC_CACHE[lp] = build_nc(lp)
    res = run_bass_kernel_spmd(_NC_CACHE[lp], in_maps, list(range(NCORES)))
    out = np.concatenate([r["out"].reshape(BC, S, 1) for r in res.results],
                         axis=0)
    return out.astype(f32)


if __name__ == "__main__":
    pass


# revision 13
# speedup vs baseline: 2.2001x; 1.1164x over previous
"""Trainium2 Bass kernel for nn_ListenerModel (scatter_memory).

Strategy: pure data-parallel over batch (B=64 -> 8 rows/core), weights
replicated.  Key points:
  - masked sequence positions have softmax weight exactly 0, so the
    host compacts each batch's sequence to its unmasked positions
    (variable per-batch length lp_b, rounded to 8).  Math is exact;
    pad slots get -1e30 mask.  This cuts the dominant matmul chain and
    reps DMA ~2x.
  - all matmul operands bf16 (fp32r streams 2 cyc/col; bf16 1).
  - host lays every tensor out partition-major so DMAs are contiguous;
    two queues (sync: start-critical + activations, gpsimd: W_vis
    stream + late weights) ordered by need-time; reps pool holds all
    8 batches so no DMA ever waits on compute.
  - visual-context matmuls (M=8) packed 4-wide into PE column groups,
    interleaved into mm1 against modeled DMA arrival times.
  - per-b softmax/attend runs several blocks behind the matmul stream;
    sep/history fillers are interleaved between early per-b blocks
    (their data arrives on the queue tails); outputs produced per-b
    via attcol x sepfinT matmuls (no serial transpose tail).
"""

import numpy as np
import ml_dtypes
from contextlib import ExitStack

import concourse.bass as bass
import concourse.mybir as mybir
from concourse import bacc, tile
from concourse.bass_utils import run_bass_kernel_spmd

NCORES = 8
B, L, S, H = 64, 512, 6, 8
EMBED, HID, IMG, ATT = 1024, 512, 2048, 256
SIMG = S * IMG          # 12288
BC = B // NCORES        # 8 batch rows per core
BS = BC * S             # 48 (b,s) rows per core
BSH = BS * H            # 384
P = 128
FP = mybir.dt.float32
BF = mybir.dt.bfloat16

KE = EMBED // P         # 8  k-chunks for EMBED contraction
KH = HID // P           # 4  k-chunks for HID contraction
KA = ATT // P           # 2  k-chunks for ATT contraction
KV = SIMG // P          # 96 k-chunks for the visual-context matmul
KI = IMG // P           # 16 k-chunks for separate-image projection
KBH = BSH // P          # 3  k-chunks for history averaging
NHT = HID // P          # 4  hid tiles
NAT = ATT // P          # 2  att tiles

WVB = 4                 # W_vis chunks per DMA / per packed vc group
NVG = KV // WVB         # 24 vc chunk groups


def build_nc(lps):
    """lps: per-core tuple of BC compacted sequence lengths."""
    lmax = max(lps)
    nc = bacc.Bacc(None)

    d_reps = nc.dram_tensor("repsT", [BC, P, KE * lmax], BF,
                            kind="ExternalInput")
    d_vcT = nc.dram_tensor("vcT", [P, KV * BC], BF, kind="ExternalInput")
    d_sepT = nc.dram_tensor("sepT", [P, KI * BS], BF, kind="ExternalInput")
    d_hist = nc.dram_tensor("histf", [P, KBH * EMBED], BF, kind="ExternalInput")
    d_validW = nc.dram_tensor("validW", [P, KBH * BS], BF, kind="ExternalInput")
    d_Wvis = nc.dram_tensor("Wvis", [NVG, P, WVB * HID], BF, kind="ExternalInput")
    d_Wemb = nc.dram_tensor("Wemb", [P, KE * HID], BF, kind="ExternalInput")
    d_Wmm = nc.dram_tensor("Wmm", [P, 2 * KH * HID], BF, kind="ExternalInput")
    d_Wsep = nc.dram_tensor("Wsep", [P, KI * HID], BF, kind="ExternalInput")
    d_Wa1 = nc.dram_tensor("Wa1", [P, KH * ATT], BF, kind="ExternalInput")
    d_Wa2 = nc.dram_tensor("Wa2", [P, KA], BF, kind="ExternalInput")
    d_bvis = nc.dram_tensor("bvis_row", [1, HID], BF, kind="ExternalInput")
    d_bsep = nc.dram_tensor("bsep_row", [1, HID], BF, kind="ExternalInput")
    d_bemb_row = nc.dram_tensor("bemb_row", [1, HID], BF, kind="ExternalInput")
    d_ones = nc.dram_tensor("ones_row", [1, P], BF, kind="ExternalInput")
    d_bemb_col = nc.dram_tensor("bemb_col", [P, NHT], FP, kind="ExternalInput")
    d_bmm_col = nc.dram_tensor("bmm_col", [P, NHT], FP, kind="ExternalInput")
    d_ba1_col = nc.dram_tensor("ba1_col", [P, NAT], FP, kind="ExternalInput")
    d_mask = nc.dram_tensor("mask_row", [BC, lmax], FP, kind="ExternalInput")
    d_hh = nc.dram_tensor("hh_col", [BS, 1], FP, kind="ExternalInput")
    d_ident = nc.dram_tensor("ident", [P, P], FP, kind="ExternalInput")
    d_gsel = nc.dram_tensor("gsel", [P, BC], BF, kind="ExternalInput")
    d_out = nc.dram_tensor("out", [1, BS], FP, kind="ExternalOutput")

    AFT = mybir.ActivationFunctionType
    AX = mybir.AxisListType

    with ExitStack() as ctx:
        tc = ctx.enter_context(tile.TileContext(nc))
        wres = ctx.enter_context(tc.tile_pool(name="wres", bufs=1))
        repsp = ctx.enter_context(tc.tile_pool(name="repsp", bufs=8))
        wvp = ctx.enter_context(tc.tile_pool(name="wvp", bufs=4))
        mm1p = ctx.enter_context(tc.tile_pool(name="mm1p", bufs=32))
        mm2p = ctx.enter_context(tc.tile_pool(name="mm2p", bufs=24))
        atthp = ctx.enter_context(tc.tile_pool(name="atthp", bufs=4))
        tmpp = ctx.enter_context(tc.tile_pool(name="tmpp", bufs=3))
        wbtp = ctx.enter_context(tc.tile_pool(name="wbtp", bufs=2))
        attcp = ctx.enter_context(tc.tile_pool(name="attcp", bufs=8))
        smp = ctx.enter_context(tc.tile_pool(name="smp", bufs=3))
        wrp = ctx.enter_context(tc.tile_pool(name="wrp", bufs=8))
        # PSUM: 2 + 1 + 2 + 3 = 8 banks
        psM = ctx.enter_context(tc.tile_pool(name="psM", bufs=2, space="PSUM"))
        psV = ctx.enter_context(tc.tile_pool(name="psV", bufs=1, space="PSUM"))
        psE = ctx.enter_context(tc.tile_pool(name="psE", bufs=2, space="PSUM"))
        psB = ctx.enter_context(tc.tile_pool(name="psB", bufs=3, space="PSUM"))

        def wtile(shape, tag, dt=FP):
            return wres.tile(shape, dt, tag=tag, name=tag)

        def loadw(dst, src):
            nc.gpsimd.dma_start(out=dst, in_=src)

        def loads(dst, src):
            nc.sync.dma_start(out=dst, in_=src)

        def body():
            # ===== DMA queue S (sync): start-critical, then tail data =====
            wemb = wtile([P, KE, HID], "wemb", BF)
            for h in range(NHT):
                loads(wemb[:, :, h * P:(h + 1) * P],
                      d_Wemb.rearrange("p (k h) -> p k h", k=KE)
                      [:, :, h * P:(h + 1) * P])
            vct = wtile([P, KV, BC], "vct", BF)
            loads(vct, d_vcT.rearrange("p (k b) -> p k b", k=KV))
            reps_sb = []
            for b in range(BC):
                t = repsp.tile([P, KE, lps[b]], BF, tag="reps", name=f"reps{b}")
                loads(t, d_reps[b][:, :KE * lps[b]]
                      .rearrange("p (k l) -> p k l", k=KE))
                reps_sb.append(t)
            mrows = []
            for b in range(BC):
                m = wtile([1, lps[b]], f"mrow{b}")
                loads(m, d_mask[b:b + 1, :lps[b]])
                mrows.append(m)
            hh_sb = wtile([BS, 1], "hh")
            loads(hh_sb, d_hh[:, :])
            # tail: filler data, consumed between early per-b blocks
            sepT_sb = wtile([P, KI, BS], "sepT", BF)
            loads(sepT_sb, d_sepT.rearrange("p (k s) -> p k s", k=KI))
            wsep_sb = wtile([P, KI, HID], "wsep", BF)
            loads(wsep_sb, d_Wsep.rearrange("p (k h) -> p k h", k=KI))
            histf_sb = wtile([P, KBH, EMBED], "histf", BF)
            loads(histf_sb, d_hist.rearrange("p (k e) -> p k e", k=KBH))
            validW_sb = wtile([P, KBH, BS], "validW", BF)
            loads(validW_sb, d_validW.rearrange("p (k s) -> p k s", k=KBH))

            # ===== DMA queue W (gpsimd): consts, W_vis stream, weights =====
            bembc_sb = wtile([P, NHT], "bembc")
            loadw(bembc_sb, d_bemb_col[:, :])
            ones = wtile([1, P], "ones", BF)
            loadw(ones, d_ones[:, :])
            ident = wtile([P, P], "ident")
            loadw(ident, d_ident[:, :])
            gsel = wtile([P, BC], "gsel", BF)
            loadw(gsel, d_gsel[:, :])
            bvis_sb = wtile([1, HID], "bvis", BF)
            loadw(bvis_sb, d_bvis[:, :])
            bsep_sb = wtile([1, HID], "bsep", BF)
            loadw(bsep_sb, d_bsep[:, :])
            bembr_sb = wtile([1, HID], "bembr", BF)
            loadw(bembr_sb, d_bemb_row[:, :])
            ba1c_sb = wtile([P, NAT], "ba1c")
            loadw(ba1c_sb, d_ba1_col[:, :])
            bmmc_sb = wtile([P, NHT], "bmmc")
            loadw(bmmc_sb, d_bmm_col[:, :])
            wa2_sb = wtile([P, KA], "wa2", BF)
            loadw(wa2_sb, d_Wa2[:, :])
            wv_tiles = []
            for g in range(NVG):
                wv = wvp.tile([P, WVB, HID], BF, tag="wv", name="wv")
                loadw(wv, d_Wvis[g].rearrange("p (k h) -> p k h", k=WVB))
                wv_tiles.append(wv)
            wmm = wtile([P, 2 * KH, HID], "wmm", BF)
            loadw(wmm, d_Wmm.rearrange("p (k h) -> p k h", k=2 * KH))
            wa1 = wtile([P, KH, ATT], "wa1", BF)
            loadw(wa1, d_Wa1.rearrange("p (k h) -> p k h", k=KH))

            # =========== phase 1: mm1 for all b, vc interleaved ===========
            vc_ps = psV.tile([P, HID], FP, tag="V", name="vc_ps")
            nc.vector.memset(vc_ps[:, :], 0.0)

            def emit_vc_group(g):
                for j in range(WVB):
                    k = g * WVB + j
                    cg = 32 * j
                    nc.tensor.matmul(vc_ps[cg:cg + BC, :], vct[:, k, :],
                                     wv_tiles[g][:, j, :],
                                     start=(g == 0), stop=(g == NVG - 1),
                                     tile_position=(0, cg),
                                     skip_group_check=True)

            mm1_sb = {}
            vc_emitted = 0
            htile_no = 0

            def maybe_vc():
                # wvis group g lands ~(2.2g + 5)us; h-tile t ends
                # ~(0.95t + 13)us -> emit g after h-tile ~2.3g - 8
                nonlocal vc_emitted
                while (vc_emitted < NVG
                       and htile_no >= max(1, int(2.3 * vc_emitted - 8))):
                    emit_vc_group(vc_emitted)
                    vc_emitted += 1

            for b in range(BC):
                for h in range(NHT):
                    ps = psM.tile([P, lps[b]], FP, tag="M", name="mm1ps")
                    for k in range(KE):
                        nc.tensor.matmul(
                            ps[:, :],
                            wemb[:, k, h * P:(h + 1) * P],
                            reps_sb[b][:, k, :],
                            start=(k == 0), stop=(k == KE - 1))
                    t = mm1p.tile([P, lps[b]], BF, tag="mm1",
                                  name=f"mm1_{b}_{h}")
                    nc.scalar.activation(t, ps[:, :], AFT.Relu,
                                         bias=bembc_sb[:, h:h + 1])
                    mm1_sb[(b, h)] = t
                    htile_no += 1
                    maybe_vc()
            while vc_emitted < NVG:
                emit_vc_group(vc_emitted)
                vc_emitted += 1

            # ---- ctx chain ----
            vcp_sb = wtile([P, HID], "vcp_sb", BF)
            nc.scalar.activation(vcp_sb, vc_ps[:, :], AFT.Identity)
            ctx_ps = psE.tile([BC, HID], FP, tag="E", name="ctx_ps")
            nc.tensor.matmul(ctx_ps[:, :], gsel[:, :], vcp_sb[:, :],
                             start=True, stop=False)
            nc.tensor.matmul(ctx_ps[:, :], ones[:, :BC], bvis_sb[:, :],
                             start=False, stop=True)
            ctx_sb = wtile([BC, HID], "ctx_sb")
            nc.scalar.activation(ctx_sb, ctx_ps[:, :], AFT.Relu)
            ctxT_sb = [wtile([P, BC], f"ctxT{h}", BF) for h in range(NHT)]
            for h in range(NHT):
                tp = psE.tile([P, BC], FP, tag="E", name="ctxT_ps")
                nc.tensor.transpose(tp[:, :], ctx_sb[:, h * P:(h + 1) * P],
                                    ident[:BC, :BC])
                nc.scalar.activation(ctxT_sb[h], tp[:, :], AFT.Identity)
            ctxmmb_sb = [wtile([P, BC], f"ctxmmb{h}") for h in range(NHT)]
            for h2 in range(NHT):
                ps = psE.tile([P, BC], FP, tag="E", name="ctxmm_ps")
                for k in range(KH):
                    nc.tensor.matmul(ps[:, :],
                                     wmm[:, KH + k, h2 * P:(h2 + 1) * P],
                                     ctxT_sb[k][:, :],
                                     start=(k == 0), stop=(k == KH - 1))
                nc.scalar.activation(ctxmmb_sb[h2], ps[:, :], AFT.Identity,
                                     bias=bmmc_sb[:, h2:h2 + 1])

            # =========== phase 2: per-b pipeline with fillers ===========
            outrow = wtile([1, BS], "outrow")
            wrow_q = {}
            mm2_q = {}
            sep_sb = wtile([BS, HID], "sep_sb")
            hadd_sb = wtile([BS, HID], "hadd_sb")
            sepfin_sb = wtile([BS, HID], "sepfin_sb")
            sepfinT = [wtile([P, BS], f"sepfinT{h}", BF) for h in range(NHT)]
            havgT_sb = [wtile([P, BS], f"havgT{e}", BF) for e in range(KE)]

            def fill_sep():
                sep_ps = psE.tile([BS, HID], FP, tag="E", name="sep_ps")
                for k in range(KI):
                    nc.tensor.matmul(sep_ps[:, :], sepT_sb[:, k, :],
                                     wsep_sb[:, k, :],
                                     start=(k == 0), stop=False)
                nc.tensor.matmul(sep_ps[:, :], ones[:, :BS], bsep_sb[:, :],
                                 start=False, stop=True)
                nc.vector.tensor_copy(sep_sb, sep_ps[:, :])

            def fill_havg():
                for e in range(KE):
                    ps = psE.tile([P, BS], FP, tag="E", name="havg_ps")
                    for k in range(KBH):
                        nc.tensor.matmul(ps[:, :],
                                         histf_sb[:, k, e * P:(e + 1) * P],
                                         validW_sb[:, k, :],
                                         start=(k == 0), stop=(k == KBH - 1))
                    nc.scalar.activation(havgT_sb[e], ps[:, :], AFT.Identity)

            def fill_ha():
                ha_ps = psE.tile([BS, HID], FP, tag="E", name="ha_ps")
                for e in range(KE):
                    nc.tensor.matmul(ha_ps[:, :], havgT_sb[e][:, :],
                                     wemb[:, e, :],
                                     start=(e == 0), stop=False)
                nc.tensor.matmul(ha_ps[:, :], ones[:, :BS], bembr_sb[:, :],
                                 start=False, stop=True)
                nc.scalar.activation(hadd_sb, ha_ps[:, :], AFT.Relu)

            def fill_sepfin():
                nc.vector.tensor_scalar_mul(sepfin_sb, hadd_sb, hh_sb)
                nc.vector.tensor_add(sepfin_sb, sepfin_sb, sep_sb)
                for h in range(NHT):
                    tp = psE.tile([P, BS], FP, tag="E", name="sfT_ps")
                    nc.tensor.transpose(tp[:, :],
                                        sepfin_sb[:, h * P:(h + 1) * P],
                                        ident[:BS, :BS])
                    nc.scalar.activation(sepfinT[h], tp[:, :], AFT.Identity)

            def emit_attend(b):
                lp = lps[b]
                wb_ps = psB.tile([P, lp], FP, tag="B", name="wbps")
                nc.tensor.matmul(wb_ps[:, :], ones[:, :], wrow_q.pop(b)[:, :],
                                 start=True, stop=True)
                wbt = wbtp.tile([P, lp], BF, tag="wbt", name="wbt")
                nc.vector.tensor_copy(wbt, wb_ps[:, :])
                attc = []
                for h2 in range(NHT):
                    tmp = tmpp.tile([P, lp], BF, tag="tmpa", name="tmpa")
                    nc.vector.tensor_mul(tmp, mm2_q[b][h2][:, :], wbt)
                    ac = attcp.tile([P, 1], BF, tag="attc", name="attc")
                    with nc.allow_low_precision(
                            reason="attended col consumed by bf16 matmul"):
                        nc.vector.reduce_sum(ac, tmp, axis=AX.X)
                    attc.append(ac)
                del mm2_q[b]
                o_ps = psE.tile([1, BS], FP, tag="E", name="o_ps")
                for h2 in range(NHT):
                    nc.tensor.matmul(o_ps[:, :], attc[h2][:, :],
                                     sepfinT[h2][:, :],
                                     start=(h2 == 0), stop=(h2 == NHT - 1))
                nc.vector.tensor_copy(outrow[0:1, S * b:S * (b + 1)],
                                      o_ps[0:1, S * b:S * (b + 1)])

            def emit_block(b):
                lp = lps[b]
                mm2t = []
                for h2 in range(NHT):
                    ps = psB.tile([P, lp], FP, tag="B", name="mm2ps")
                    for k in range(KH):
                        nc.tensor.matmul(ps[:, :],
                                         wmm[:, k, h2 * P:(h2 + 1) * P],
                                         mm1_sb[(b, k)][:, :],
                                         start=(k == 0), stop=(k == KH - 1))
                    t = mm2p.tile([P, lp], BF, tag="mm2", name="mm2t")
                    nc.scalar.activation(t, ps[:, :], AFT.Relu,
                                         bias=ctxmmb_sb[h2][:, b:b + 1])
                    mm2t.append(t)
                mm2_q[b] = mm2t
                atth = []
                for a in range(NAT):
                    ps = psB.tile([P, lp], FP, tag="B", name="mm3ps")
                    for k in range(KH):
                        nc.tensor.matmul(ps[:, :],
                                         wa1[:, k, a * P:(a + 1) * P],
                                         mm2t[k][:, :],
                                         start=(k == 0), stop=(k == KH - 1))
                    t = atthp.tile([P, lp], BF, tag="atth", name="atht")
                    nc.scalar.activation(t, ps[:, :], AFT.Tanh,
                                         bias=ba1c_sb[:, a:a + 1])
                    atth.append(t)
                sc_ps = psB.tile([1, lp], FP, tag="B", name="scps")
                for k in range(KA):
                    nc.tensor.matmul(sc_ps[:, :], wa2_sb[:, k:k + 1],
                                     atth[k][:, :],
                                     start=(k == 0), stop=(k == KA - 1))
                att_row = smp.tile([1, lp], FP, tag="attrow", name="att_row")
                nc.vector.tensor_add(att_row, sc_ps[:, :], mrows[b])
                negmax = smp.tile([1, 1], FP, tag="negmax", name="negmax")
                nc.vector.reduce_max(negmax, att_row, axis=AX.X, negate=True)
                esum = smp.tile([1, 1], FP, tag="esum", name="esum")
                nc.scalar.activation(att_row, att_row, AFT.Exp, bias=negmax,
                                     accum_out=esum)
                rec = smp.tile([1, 1], FP, tag="rec", name="rec")
                nc.vector.reciprocal(rec, esum)
                wrow = wrp.tile([1, lp], BF, tag="wrow", name="wrow")
                nc.scalar.activation(wrow, att_row, AFT.Copy, scale=rec)
                wrow_q[b] = wrow

            emit_block(0)
            emit_block(1)
            fill_sep()
            emit_block(2)
            fill_havg()
            emit_block(3)
            fill_ha()
            emit_block(4)
            fill_sepfin()
            emit_block(5)
            emit_attend(0)
            emit_attend(1)
            emit_block(6)
            emit_attend(2)
            emit_attend(3)
            emit_attend(4)
            emit_block(7)
            emit_attend(5)
            emit_attend(6)
            emit_attend(7)

            nc.sync.dma_start(out=d_out[:, :], in_=outrow)

        body()

    nc.compile()
    return nc


_NC_CACHE = {}


def kernel(reps, separate_imgs, visual_context, masks, hist, hist_len,
           W_vis, b_vis, W_emb, b_emb, W_mm, b_mm, W_sep, b_sep,
           W_a1, b_a1, W_a2, b_a2):
    f32 = np.float32
    bf16 = ml_dtypes.bfloat16

    def pm(a, kchunks):
        """[K, W] -> partition-major bf16 [128, kchunks*W]."""
        a = np.ascontiguousarray(a, f32)
        K, W = a.shape
        assert K == kchunks * P
        out = a.reshape(kchunks, P, W).transpose(1, 0, 2)
        return np.ascontiguousarray(out).astype(bf16).reshape(P, kchunks * W)

    reps = np.asarray(reps, f32)
    separate_imgs = np.asarray(separate_imgs, f32)
    visual_context = np.asarray(visual_context, f32)
    hist = np.asarray(hist, f32)
    hist_len = np.asarray(hist_len, np.int32)
    masks = np.asarray(masks)[:, :, 0]          # True -> masked out

    # ---- compact each batch's sequence to its unmasked positions ----
    # all cores run one SPMD program, so slot b's capacity is the max
    # keep-count over cores at that position (rounded up to 8)
    keep_idx = [np.nonzero(~masks[b])[0] for b in range(B)]
    prog_lps = tuple(
        min(max((max(len(keep_idx[c * BC + b]) for c in range(NCORES))
                 + 7) // 8 * 8, 8), L)
        for b in range(BC))
    lmax_all = max(prog_lps)

    ident = np.eye(P, dtype=f32)
    gsel = np.zeros((P, BC), f32)
    for j in range(4):
        for i in range(BC):
            gsel[32 * j + i, i] = 1.0

    wvis_pm = np.ascontiguousarray(
        np.asarray(W_vis, f32).reshape(NVG, WVB, P, HID).transpose(0, 2, 1, 3)
    ).astype(bf16).reshape(NVG, P, WVB * HID)

    shared = {
        "Wvis": wvis_pm,
        "Wemb": pm(np.asarray(W_emb, f32), KE),
        "Wmm": pm(np.asarray(W_mm, f32), 2 * KH),
        "Wsep": pm(np.asarray(W_sep, f32), KI),
        "Wa1": pm(np.asarray(W_a1, f32), KH),
        "Wa2": pm(np.asarray(W_a2, f32).reshape(ATT, 1), KA).reshape(P, KA),
        "bvis_row": np.asarray(b_vis, f32).reshape(1, HID).astype(bf16),
        "bsep_row": np.asarray(b_sep, f32).reshape(1, HID).astype(bf16),
        "bemb_row": np.asarray(b_emb, f32).reshape(1, HID).astype(bf16),
        "bemb_col": np.ascontiguousarray(
            np.asarray(b_emb, f32).reshape(NHT, P).T),
        "bmm_col": np.ascontiguousarray(
            np.asarray(b_mm, f32).reshape(NHT, P).T),
        "ba1_col": np.ascontiguousarray(
            np.asarray(b_a1, f32).reshape(NAT, P).T),
        "ones_row": np.ones((1, P), bf16),
        "ident": ident,
        "gsel": gsel.astype(bf16),
    }

    in_maps = []
    for c in range(NCORES):
        sl = slice(c * BC, (c + 1) * BC)
        repsT = np.zeros((BC, P, KE * lmax_all), bf16)
        mask_c = np.zeros((BC, lmax_all), f32)
        for b in range(BC):
            gb = c * BC + b
            ix = keep_idx[gb]
            lp = prog_lps[b]
            r = np.zeros((lp, EMBED), f32)
            r[:len(ix)] = reps[gb, ix]
            rpm = r.reshape(lp, KE, P).transpose(2, 1, 0)  # [P, KE, lp]
            repsT[b, :, :KE * lp] = np.ascontiguousarray(rpm) \
                .astype(bf16).reshape(P, KE * lp)
            mask_c[b, :lp] = f32(-1e30)
            mask_c[b, :len(ix)] = 0.0
        mask_c += f32(b_a2[0])

        hl = hist_len[sl].reshape(BS)
        hvalid = (np.arange(H)[None, :] < hl[:, None]).astype(f32)
        hvalid /= np.maximum(hl, 1).astype(f32)[:, None]
        validW = np.zeros((BSH, BS), f32)
        for bs in range(BS):
            validW[bs * H:(bs + 1) * H, bs] = hvalid[bs]
        vcT = visual_context[sl].reshape(BC, KV, P).transpose(2, 1, 0)
        sepT = separate_imgs[sl].reshape(BS, KI, P).transpose(2, 1, 0)
        m = {
            "repsT": repsT,
            "vcT": np.ascontiguousarray(vcT).astype(bf16).reshape(P, KV * BC),
            "sepT": np.ascontiguousarray(sepT).astype(bf16)
                      .reshape(P, KI * BS),
            "histf": pm(hist[sl].reshape(BSH, EMBED), KBH),
            "validW": pm(validW, KBH),
            "mask_row": mask_c,
            "hh_col": (hl > 0).astype(f32).reshape(BS, 1),
        }
        m.update(shared)
        in_maps.append(m)

    if prog_lps not in _NC_CACHE:
        _NC_CACHE[prog_lps] = build_nc(prog_lps)
    res = run_bass_kernel_spmd(_NC_CACHE[prog_lps], in_maps,
                               list(range(NCORES)))
    out = np.concatenate([r["out"].reshape(BC, S, 1) for r in res.results],
                         axis=0)
    return out.astype(f32)


if __name__ == "__main__":
    pass


# revision 21
# speedup vs baseline: 2.4056x; 1.0934x over previous
"""Trainium2 Bass kernel for nn_ListenerModel (scatter_memory).

Strategy: pure data-parallel over batch (B=64 -> 8 rows/core), weights
replicated.  Key points:
  - masked sequence positions have softmax weight exactly 0, so the
    host compacts each batch's sequence to its unmasked positions
    (variable per-batch length lp_b, rounded to 8).  Math is exact;
    pad slots get -1e30 mask.  This cuts the dominant matmul chain and
    reps DMA ~2x.
  - all matmul operands bf16 (fp32r streams 2 cyc/col; bf16 1).
  - host lays every tensor out partition-major so DMAs are contiguous;
    two queues (sync: start-critical + activations, gpsimd: W_vis
    stream + late weights) ordered by need-time; reps pool holds all
    8 batches so no DMA ever waits on compute.
  - visual-context matmuls (M=8) packed 4-wide into PE column groups,
    interleaved into mm1 against modeled DMA arrival times.
  - per-b softmax/attend runs several blocks behind the matmul stream;
    sep/history fillers are interleaved between early per-b blocks
    (their data arrives on the queue tails); outputs produced per-b
    via attcol x sepfinT matmuls (no serial transpose tail).
"""

import numpy as np
import ml_dtypes
from contextlib import ExitStack

import concourse.bass as bass
import concourse.mybir as mybir
from concourse import bacc, tile
from concourse.bass_utils import run_bass_kernel_spmd

NCORES = 8
B, L, S, H = 64, 512, 6, 8
EMBED, HID, IMG, ATT = 1024, 512, 2048, 256
SIMG = S * IMG          # 12288
BC = B // NCORES        # 8 batch rows per core
BS = BC * S             # 48 (b,s) rows per core
BSH = BS * H            # 384
P = 128
FP = mybir.dt.float32
BF = mybir.dt.bfloat16

KE = EMBED // P         # 8  k-chunks for EMBED contraction
KH = HID // P           # 4  k-chunks for HID contraction
KA = ATT // P           # 2  k-chunks for ATT contraction
KV = SIMG // P          # 96 k-chunks for the visual-context matmul
KI = IMG // P           # 16 k-chunks for separate-image projection
KBH = BSH // P          # 3  k-chunks for history averaging
NHT = HID // P          # 4  hid tiles
NAT = ATT // P          # 2  att tiles

WVB = 4                 # W_vis chunks per DMA / per packed vc group
NVG = KV // WVB         # 24 vc chunk groups


def build_nc(lps):
    """lps: per-core tuple of BC compacted sequence lengths."""
    lmax = max(lps)
    nc = bacc.Bacc(None)

    d_reps = nc.dram_tensor("repsT", [BC, P, KE * lmax], BF,
                            kind="ExternalInput")
    d_vcT = nc.dram_tensor("vcT", [P, KV * BC], BF, kind="ExternalInput")
    d_sepT = nc.dram_tensor("sepT", [P, KI * BS], BF, kind="ExternalInput")
    d_hist = nc.dram_tensor("histf", [P, KBH * EMBED], BF, kind="ExternalInput")
    d_validW = nc.dram_tensor("validW", [P, KBH * BS], BF, kind="ExternalInput")
    d_Wvis = nc.dram_tensor("Wvis", [NVG, P, WVB * HID], BF, kind="ExternalInput")
    d_Wemb = nc.dram_tensor("Wemb", [P, KE * HID], BF, kind="ExternalInput")
    d_Wmm = nc.dram_tensor("Wmm", [P, 2 * KH * HID], BF, kind="ExternalInput")
    d_Wsep = nc.dram_tensor("Wsep", [P, KI * HID], BF, kind="ExternalInput")
    d_Wa1 = nc.dram_tensor("Wa1", [P, KH * ATT], BF, kind="ExternalInput")
    d_Wa2 = nc.dram_tensor("Wa2", [P, KA], BF, kind="ExternalInput")
    d_bvis = nc.dram_tensor("bvis_row", [1, HID], BF, kind="ExternalInput")
    d_bsep = nc.dram_tensor("bsep_row", [1, HID], BF, kind="ExternalInput")
    d_bemb_row = nc.dram_tensor("bemb_row", [1, HID], BF, kind="ExternalInput")
    d_ones = nc.dram_tensor("ones_row", [1, P], BF, kind="ExternalInput")
    d_bemb_col = nc.dram_tensor("bemb_col", [P, NHT], FP, kind="ExternalInput")
    d_bmm_col = nc.dram_tensor("bmm_col", [P, NHT], FP, kind="ExternalInput")
    d_ba1_col = nc.dram_tensor("ba1_col", [P, NAT], FP, kind="ExternalInput")
    d_mask = nc.dram_tensor("mask_row", [BC, lmax], FP, kind="ExternalInput")
    d_hh = nc.dram_tensor("hh_col", [BS, 1], FP, kind="ExternalInput")
    d_ident = nc.dram_tensor("ident", [P, P], FP, kind="ExternalInput")
    d_gsel = nc.dram_tensor("gsel", [P, BC], BF, kind="ExternalInput")
    d_out = nc.dram_tensor("out", [1, BS], FP, kind="ExternalOutput")

    AFT = mybir.ActivationFunctionType
    AX = mybir.AxisListType

    with ExitStack() as ctx:
        tc = ctx.enter_context(tile.TileContext(nc))
        wres = ctx.enter_context(tc.tile_pool(name="wres", bufs=1))
        repsp = ctx.enter_context(tc.tile_pool(name="repsp", bufs=8))
        wvp = ctx.enter_context(tc.tile_pool(name="wvp", bufs=4))
        mm1p = ctx.enter_context(tc.tile_pool(name="mm1p", bufs=32))
        mm2p = ctx.enter_context(tc.tile_pool(name="mm2p", bufs=20))
        atthp = ctx.enter_context(tc.tile_pool(name="atthp", bufs=4))
        tmpp = ctx.enter_context(tc.tile_pool(name="tmpp", bufs=3))
        wbtp = ctx.enter_context(tc.tile_pool(name="wbtp", bufs=2))
        attcp = ctx.enter_context(tc.tile_pool(name="attcp", bufs=8))
        smp = ctx.enter_context(tc.tile_pool(name="smp", bufs=3))
        wrp = ctx.enter_context(tc.tile_pool(name="wrp", bufs=8))
        # PSUM: 2 + 1 + 2 + 3 = 8 banks
        psM = ctx.enter_context(tc.tile_pool(name="psM", bufs=2, space="PSUM"))
        psV = ctx.enter_context(tc.tile_pool(name="psV", bufs=1, space="PSUM"))
        psE = ctx.enter_context(tc.tile_pool(name="psE", bufs=2, space="PSUM"))
        psB = ctx.enter_context(tc.tile_pool(name="psB", bufs=3, space="PSUM"))

        def wtile(shape, tag, dt=FP):
            return wres.tile(shape, dt, tag=tag, name=tag)

        def loadw(dst, src):
            nc.gpsimd.dma_start(out=dst, in_=src)

        def loads(dst, src):
            nc.sync.dma_start(out=dst, in_=src)

        def body():
            # ===== DMA queue S (sync): start-critical, then history =====
            wemb = wtile([P, KE, HID], "wemb", BF)
            loads(wemb, d_Wemb.rearrange("p (k h) -> p k h", k=KE))
            vct = wtile([P, KV, BC], "vct", BF)
            loads(vct, d_vcT.rearrange("p (k b) -> p k b", k=KV))
            reps_sb = []
            for b in range(BC):
                t = repsp.tile([P, KE, lps[b]], BF, tag="reps", name=f"reps{b}")
                loads(t, d_reps[b][:, :KE * lps[b]]
                      .rearrange("p (k l) -> p k l", k=KE))
                reps_sb.append(t)
            mrows = []
            for b in range(BC):
                m = wtile([1, lps[b]], f"mrow{b}")
                loads(m, d_mask[b:b + 1, :lps[b]])
                mrows.append(m)
            hh_sb = wtile([BS, 1], "hh")
            loads(hh_sb, d_hh[:, :])
            # S tail: history data (fills the pre-ctx PE window)
            histf_sb = wtile([P, KBH, EMBED], "histf", BF)
            loads(histf_sb, d_hist.rearrange("p (k e) -> p k e", k=KBH))
            validW_sb = wtile([P, KBH, BS], "validW", BF)
            loads(validW_sb, d_validW.rearrange("p (k s) -> p k s", k=KBH))

            # ===== DMA queue W (gpsimd): consts, W_vis stream, weights =====
            bembc_sb = wtile([P, NHT], "bembc")
            loadw(bembc_sb, d_bemb_col[:, :])
            ones = wtile([1, P], "ones", BF)
            loadw(ones, d_ones[:, :])
            ident = wtile([P, P], "ident")
            loadw(ident, d_ident[:, :])
            gsel = wtile([P, BC], "gsel", BF)
            loadw(gsel, d_gsel[:, :])
            bvis_sb = wtile([1, HID], "bvis", BF)
            loadw(bvis_sb, d_bvis[:, :])
            bsep_sb = wtile([1, HID], "bsep", BF)
            loadw(bsep_sb, d_bsep[:, :])
            bembr_sb = wtile([1, HID], "bembr", BF)
            loadw(bembr_sb, d_bemb_row[:, :])
            ba1c_sb = wtile([P, NAT], "ba1c")
            loadw(ba1c_sb, d_ba1_col[:, :])
            bmmc_sb = wtile([P, NHT], "bmmc")
            loadw(bmmc_sb, d_bmm_col[:, :])
            wa2_sb = wtile([P, KA], "wa2", BF)
            loadw(wa2_sb, d_Wa2[:, :])
            wv_tiles = []
            for g in range(NVG):
                wv = wvp.tile([P, WVB, HID], BF, tag="wv", name="wv")
                loadw(wv, d_Wvis[g].rearrange("p (k h) -> p k h", k=WVB))
                wv_tiles.append(wv)
            wmm = wtile([P, 2 * KH, HID], "wmm", BF)
            loadw(wmm, d_Wmm.rearrange("p (k h) -> p k h", k=2 * KH))
            wa1 = wtile([P, KH, ATT], "wa1", BF)
            loadw(wa1, d_Wa1.rearrange("p (k h) -> p k h", k=KH))
            # W tail: sep data, consumed between early per-b blocks
            wsep_sb = wtile([P, KI, HID], "wsep", BF)
            loadw(wsep_sb, d_Wsep.rearrange("p (k h) -> p k h", k=KI))
            sepT_sb = wtile([P, KI, BS], "sepT", BF)
            loadw(sepT_sb, d_sepT.rearrange("p (k s) -> p k s", k=KI))

            # =========== phase 1: mm1 for all b, vc interleaved ===========
            vc_ps = psV.tile([P, HID], FP, tag="V", name="vc_ps")
            nc.vector.memset(vc_ps[:, :], 0.0)

            def emit_vc_group(g):
                for j in range(WVB):
                    k = g * WVB + j
                    cg = 32 * j
                    nc.tensor.matmul(vc_ps[cg:cg + BC, :], vct[:, k, :],
                                     wv_tiles[g][:, j, :],
                                     start=(g == 0), stop=(g == NVG - 1),
                                     tile_position=(0, cg),
                                     skip_group_check=True)

            mm1_sb = {}
            vc_emitted = 0
            htile_no = 0

            def maybe_vc():
                # wvis group g lands ~(3.5 + 2.9g)us early, faster once the
                # sync queue drains; h-tile t ends ~(0.95t + 13)us
                nonlocal vc_emitted
                while (vc_emitted < 18
                       and htile_no >= min(31, 6 + int(1.4 * vc_emitted))):
                    emit_vc_group(vc_emitted)
                    vc_emitted += 1

            for b in range(BC):
                for h in range(NHT):
                    ps = psM.tile([P, lps[b]], FP, tag="M", name="mm1ps")
                    for k in range(KE):
                        nc.tensor.matmul(
                            ps[:, :],
                            wemb[:, k, h * P:(h + 1) * P],
                            reps_sb[b][:, k, :],
                            start=(k == 0), stop=(k == KE - 1))
                    t = mm1p.tile([P, lps[b]], BF, tag="mm1",
                                  name=f"mm1_{b}_{h}")
                    nc.scalar.activation(t, ps[:, :], AFT.Relu,
                                         bias=bembc_sb[:, h:h + 1])
                    mm1_sb[(b, h)] = t
                    htile_no += 1
                    maybe_vc()
            # =========== phase 2: per-b pipeline with fillers ===========
            ctxmmb_sb = [wtile([P, BC], f"ctxmmb{h}") for h in range(NHT)]

            def emit_ctx():
                vcp_sb = wtile([P, HID], "vcp_sb", BF)
                nc.scalar.activation(vcp_sb, vc_ps[:, :], AFT.Identity)
                ctx_ps = psE.tile([BC, HID], FP, tag="E", name="ctx_ps")
                nc.tensor.matmul(ctx_ps[:, :], gsel[:, :], vcp_sb[:, :],
                                 start=True, stop=False)
                nc.tensor.matmul(ctx_ps[:, :], ones[:, :BC], bvis_sb[:, :],
                                 start=False, stop=True)
                ctx_sb = wtile([BC, HID], "ctx_sb")
                nc.scalar.activation(ctx_sb, ctx_ps[:, :], AFT.Relu)
                ctxT_sb = [wtile([P, BC], f"ctxT{h}", BF) for h in range(NHT)]
                for h in range(NHT):
                    tp = psE.tile([P, BC], FP, tag="E", name="ctxT_ps")
                    nc.tensor.transpose(tp[:, :],
                                        ctx_sb[:, h * P:(h + 1) * P],
                                        ident[:BC, :BC])
                    nc.scalar.activation(ctxT_sb[h], tp[:, :], AFT.Identity)
                for h2 in range(NHT):
                    ps = psE.tile([P, BC], FP, tag="E", name="ctxmm_ps")
                    for k in range(KH):
                        nc.tensor.matmul(ps[:, :],
                                         wmm[:, KH + k, h2 * P:(h2 + 1) * P],
                                         ctxT_sb[k][:, :],
                                         start=(k == 0), stop=(k == KH - 1))
                    nc.scalar.activation(ctxmmb_sb[h2], ps[:, :],
                                         AFT.Identity,
                                         bias=bmmc_sb[:, h2:h2 + 1])

            outrow = wtile([1, BS], "outrow")
            wrow_q = {}
            mm2_q = {}
            sep_sb = wtile([BS, HID], "sep_sb")
            hadd_sb = wtile([BS, HID], "hadd_sb")
            sepfin_sb = wtile([BS, HID], "sepfin_sb")
            sepfinT = [wtile([P, BS], f"sepfinT{h}", BF) for h in range(NHT)]
            havgT_sb = [wtile([P, BS], f"havgT{e}", BF) for e in range(KE)]

            def fill_sep():
                sep_ps = psE.tile([BS, HID], FP, tag="E", name="sep_ps")
                for k in range(KI):
                    nc.tensor.matmul(sep_ps[:, :], sepT_sb[:, k, :],
                                     wsep_sb[:, k, :],
                                     start=(k == 0), stop=False)
                nc.tensor.matmul(sep_ps[:, :], ones[:, :BS], bsep_sb[:, :],
                                 start=False, stop=True)
                nc.vector.tensor_copy(sep_sb, sep_ps[:, :])

            def fill_havg():
                for e in range(KE):
                    ps = psE.tile([P, BS], FP, tag="E", name="havg_ps")
                    for k in range(KBH):
                        nc.tensor.matmul(ps[:, :],
                                         histf_sb[:, k, e * P:(e + 1) * P],
                                         validW_sb[:, k, :],
                                         start=(k == 0), stop=(k == KBH - 1))
                    nc.scalar.activation(havgT_sb[e], ps[:, :], AFT.Identity)

            def fill_ha():
                ha_ps = psE.tile([BS, HID], FP, tag="E", name="ha_ps")
                for e in range(KE):
                    nc.tensor.matmul(ha_ps[:, :], havgT_sb[e][:, :],
                                     wemb[:, e, :],
                                     start=(e == 0), stop=False)
                nc.tensor.matmul(ha_ps[:, :], ones[:, :BS], bembr_sb[:, :],
                                 start=False, stop=True)
                nc.scalar.activation(hadd_sb, ha_ps[:, :], AFT.Relu)

            def fill_sepfin():
                nc.vector.tensor_scalar_mul(sepfin_sb, hadd_sb, hh_sb)
                nc.vector.tensor_add(sepfin_sb, sepfin_sb, sep_sb)
                for h in range(NHT):
                    tp = psE.tile([P, BS], FP, tag="E", name="sfT_ps")
                    nc.tensor.transpose(tp[:, :],
                                        sepfin_sb[:, h * P:(h + 1) * P],
                                        ident[:BS, :BS])
                    nc.scalar.activation(sepfinT[h], tp[:, :], AFT.Identity)

            def emit_attend(b):
                lp = lps[b]
                wb_ps = psB.tile([P, lp], FP, tag="B", name="wbps")
                nc.tensor.matmul(wb_ps[:, :], ones[:, :], wrow_q.pop(b)[:, :],
                                 start=True, stop=True)
                wbt = wbtp.tile([P, lp], BF, tag="wbt", name="wbt")
                nc.vector.tensor_copy(wbt, wb_ps[:, :])
                attc = []
                for h2 in range(NHT):
                    tmp = tmpp.tile([P, lp], BF, tag="tmpa", name="tmpa")
                    nc.vector.tensor_mul(tmp, mm2_q[b][h2][:, :], wbt)
                    ac = attcp.tile([P, 1], BF, tag="attc", name="attc")
                    with nc.allow_low_precision(
                            reason="attended col consumed by bf16 matmul"):
                        nc.vector.reduce_sum(ac, tmp, axis=AX.X)
                    attc.append(ac)
                del mm2_q[b]
                o_ps = psE.tile([1, BS], FP, tag="E", name="o_ps")
                for h2 in range(NHT):
                    nc.tensor.matmul(o_ps[:, :], attc[h2][:, :],
                                     sepfinT[h2][:, :],
                                     start=(h2 == 0), stop=(h2 == NHT - 1))
                nc.vector.tensor_copy(outrow[0:1, S * b:S * (b + 1)],
                                      o_ps[0:1, S * b:S * (b + 1)])

            def emit_block(b):
                lp = lps[b]
                mm2t = []
                for h2 in range(NHT):
                    ps = psB.tile([P, lp], FP, tag="B", name="mm2ps")
                    for k in range(KH):
                        nc.tensor.matmul(ps[:, :],
                                         wmm[:, k, h2 * P:(h2 + 1) * P],
                                         mm1_sb[(b, k)][:, :],
                                         start=(k == 0), stop=(k == KH - 1))
                    t = mm2p.tile([P, lp], BF, tag="mm2", name="mm2t")
                    nc.scalar.activation(t, ps[:, :], AFT.Relu,
                                         bias=ctxmmb_sb[h2][:, b:b + 1])
                    mm2t.append(t)
                mm2_q[b] = mm2t
                atth = []
                for a in range(NAT):
                    ps = psB.tile([P, lp], FP, tag="B", name="mm3ps")
                    for k in range(KH):
                        nc.tensor.matmul(ps[:, :],
                                         wa1[:, k, a * P:(a + 1) * P],
                                         mm2t[k][:, :],
                                         start=(k == 0), stop=(k == KH - 1))
                    t = atthp.tile([P, lp], BF, tag="atth", name="atht")
                    nc.scalar.activation(t, ps[:, :], AFT.Tanh,
                                         bias=ba1c_sb[:, a:a + 1])
                    atth.append(t)
                sc_ps = psB.tile([1, lp], FP, tag="B", name="scps")
                for k in range(KA):
                    nc.tensor.matmul(sc_ps[:, :], wa2_sb[:, k:k + 1],
                                     atth[k][:, :],
                                     start=(k == 0), stop=(k == KA - 1))
                att_row = smp.tile([1, lp], FP, tag="attrow", name="att_row")
                nc.vector.tensor_add(att_row, sc_ps[:, :], mrows[b])
                negmax = smp.tile([1, 1], FP, tag="negmax", name="negmax")
                nc.vector.reduce_max(negmax, att_row, axis=AX.X, negate=True)
                esum = smp.tile([1, 1], FP, tag="esum", name="esum")
                nc.scalar.activation(att_row, att_row, AFT.Exp, bias=negmax,
                                     accum_out=esum)
                rec = smp.tile([1, 1], FP, tag="rec", name="rec")
                nc.vector.reciprocal(rec, esum)
                wrow = wrp.tile([1, lp], BF, tag="wrow", name="wrow")
                nc.scalar.activation(wrow, att_row, AFT.Copy, scale=rec)
                wrow_q[b] = wrow

            # history fillers use S-queue data that lands before the
            # W_vis tail; they occupy the PE while W_vis/W_mm finish
            fill_havg()
            fill_ha()
            while vc_emitted < NVG:
                emit_vc_group(vc_emitted)
                vc_emitted += 1
            emit_ctx()
            emit_block(0)
            emit_block(1)
            emit_block(2)
            fill_sep()
            fill_sepfin()
            emit_block(3)
            emit_attend(0)
            emit_block(4)
            emit_attend(1)
            emit_block(5)
            emit_attend(2)
            emit_attend(3)
            emit_block(6)
            emit_attend(4)
            emit_attend(5)
            emit_block(7)
            emit_attend(6)
            emit_attend(7)

            nc.sync.dma_start(out=d_out[:, :], in_=outrow)

        body()

    nc.compile()
    return nc


_NC_CACHE = {}


def kernel(reps, separate_imgs, visual_context, masks, hist, hist_len,
           W_vis, b_vis, W_emb, b_emb, W_mm, b_mm, W_sep, b_sep,
           W_a1, b_a1, W_a2, b_a2):
    f32 = np.float32
    bf16 = ml_dtypes.bfloat16

    def pm(a, kchunks):
        """[K, W] -> partition-major bf16 [128, kchunks*W]."""
        a = np.ascontiguousarray(a, f32)
        K, W = a.shape
        assert K == kchunks * P
        out = a.reshape(kchunks, P, W).transpose(1, 0, 2)
        return np.ascontiguousarray(out).astype(bf16).reshape(P, kchunks * W)

    reps = np.asarray(reps, f32)
    separate_imgs = np.asarray(separate_imgs, f32)
    visual_context = np.asarray(visual_context, f32)
    hist = np.asarray(hist, f32)
    hist_len = np.asarray(hist_len, np.int32)
    masks = np.asarray(masks)[:, :, 0]          # True -> masked out

    # ---- compact each batch's sequence to its unmasked positions ----
    # all cores run one SPMD program, so slot b's capacity is the max
    # keep-count over cores at that position (rounded up to 8)
    keep_idx = [np.nonzero(~masks[b])[0] for b in range(B)]
    prog_lps = tuple(
        min(max((max(len(keep_idx[c * BC + b]) for c in range(NCORES))
                 + 7) // 8 * 8, 8), L)
        for b in range(BC))
    lmax_all = max(prog_lps)

    ident = np.eye(P, dtype=f32)
    gsel = np.zeros((P, BC), f32)
    for j in range(4):
        for i in range(BC):
            gsel[32 * j + i, i] = 1.0

    wvis_pm = np.ascontiguousarray(
        np.asarray(W_vis, f32).reshape(NVG, WVB, P, HID).transpose(0, 2, 1, 3)
    ).astype(bf16).reshape(NVG, P, WVB * HID)

    shared = {
        "Wvis": wvis_pm,
        "Wemb": pm(np.asarray(W_emb, f32), KE),
        "Wmm": pm(np.asarray(W_mm, f32), 2 * KH),
        "Wsep": pm(np.asarray(W_sep, f32), KI),
        "Wa1": pm(np.asarray(W_a1, f32), KH),
        "Wa2": pm(np.asarray(W_a2, f32).reshape(ATT, 1), KA).reshape(P, KA),
        "bvis_row": np.asarray(b_vis, f32).reshape(1, HID).astype(bf16),
        "bsep_row": np.asarray(b_sep, f32).reshape(1, HID).astype(bf16),
        "bemb_row": np.asarray(b_emb, f32).reshape(1, HID).astype(bf16),
        "bemb_col": np.ascontiguousarray(
            np.asarray(b_emb, f32).reshape(NHT, P).T),
        "bmm_col": np.ascontiguousarray(
            np.asarray(b_mm, f32).reshape(NHT, P).T),
        "ba1_col": np.ascontiguousarray(
            np.asarray(b_a1, f32).reshape(NAT, P).T),
        "ones_row": np.ones((1, P), bf16),
        "ident": ident,
        "gsel": gsel.astype(bf16),
    }

    in_maps = []
    for c in range(NCORES):
        sl = slice(c * BC, (c + 1) * BC)
        repsT = np.zeros((BC, P, KE * lmax_all), bf16)
        mask_c = np.zeros((BC, lmax_all), f32)
        for b in range(BC):
            gb = c * BC + b
            ix = keep_idx[gb]
            lp = prog_lps[b]
            r = np.zeros((lp, EMBED), f32)
            r[:len(ix)] = reps[gb, ix]
            rpm = r.reshape(lp, KE, P).transpose(2, 1, 0)  # [P, KE, lp]
            repsT[b, :, :KE * lp] = np.ascontiguousarray(rpm) \
                .astype(bf16).reshape(P, KE * lp)
            mask_c[b, :lp] = f32(-1e30)
            mask_c[b, :len(ix)] = 0.0
        mask_c += f32(b_a2[0])

        hl = hist_len[sl].reshape(BS)
        hvalid = (np.arange(H)[None, :] < hl[:, None]).astype(f32)
        hvalid /= np.maximum(hl, 1).astype(f32)[:, None]
        validW = np.zeros((BSH, BS), f32)
        for bs in range(BS):
            validW[bs * H:(bs + 1) * H, bs] = hvalid[bs]
        vcT = visual_context[sl].reshape(BC, KV, P).transpose(2, 1, 0)
        sepT = separate_imgs[sl].reshape(BS, KI, P).transpose(2, 1, 0)
        m = {
            "repsT": repsT,
            "vcT": np.ascontiguousarray(vcT).astype(bf16).reshape(P, KV * BC),
            "sepT": np.ascontiguousarray(sepT).astype(bf16)
                      .reshape(P, KI * BS),
            "histf": pm(hist[sl].reshape(BSH, EMBED), KBH),
            "validW": pm(validW, KBH),
            "mask_row": mask_c,
            "hh_col": (hl > 0).astype(f32).reshape(BS, 1),
        }
        m.update(shared)
        in_maps.append(m)

    if prog_lps not in _NC_CACHE:
        _NC_CACHE[prog_lps] = build_nc(prog_lps)
    res = run_bass_kernel_spmd(_NC_CACHE[prog_lps], in_maps,
                               list(range(NCORES)))
    out = np.concatenate([r["out"].reshape(BC, S, 1) for r in res.results],
                         axis=0)
    return out.astype(f32)


if __name__ == "__main__":
    pass
